# revision 1
# baseline (speedup 1.0000x reference)
"""MoE SwiGLU (T=4096, D=I=1024, E=8, top-2) on 8 Trainium2 NeuronCores.

Expert-parallel with on-device routing: core e holds expert e's weights
in SBUF.  The gate (scores -> softmax -> top-2) is replicated on every
core in true fp32.  Each core then COMPACTS the token ids routed to its
expert (matmul prefix-sums + indirect scatter), gathers just those x
rows (indirect DMA), computes SwiGLU only for them (float32r matmuls at
full PE rate), scales by the routing weight, and scatters the rows into
a zeroed per-range contribution buffer.  Four token-range ReduceScatters
overlap compute; the host reassembles the 8 shards.

Work is organized in 4 token ranges of 1024; per (core, range) the
routed token count is ~256 +- 14 (capacity 384, overflow checked on the
host against the actual gate before launch).
"""
import os
import sys

import numpy as np

for _p in ("/opt/trn_rl_repo", "/root/.axon_site/_ro/trn_rl_repo"):
    if os.path.isdir(_p) and _p not in sys.path:
        sys.path.append(_p)

import concourse.bass as bass  # noqa: E402
import concourse.mybir as mybir  # noqa: E402
import concourse.tile as tile  # noqa: E402
from concourse import bacc  # noqa: E402
from concourse.bass_utils import run_bass_kernel_spmd  # noqa: E402

P = 128
T, D, I, E, TOPK = 4096, 1024, 1024, 8, 2
NCORES = 8
TCH = 512            # gate token chunk (matmul free dim)
NCH = T // TCH       # 8
DK = D // P          # 8
IK = I // P          # 8
NQ = 4               # ReduceScatter ranges
RT = T // NQ         # 1024 tokens per range
RSH = RT // NCORES   # 128-token shard per core per range
CAP = 384            # routed-token capacity per (core, range)
CT = CAP // P        # 3 c-tiles per range
YC_ROWS = RT + P     # contribution rows + trash row region
XPAD_ROWS = T + P    # x padded with zero rows (gather trash target)
f32 = mybir.dt.float32
f32r = mybir.dt.float32r
i32 = mybir.dt.int32

_CACHED_NC = None


def _build():
    nc = bacc.Bacc("TRN2", target_bir_lowering=False, debug=False,
                   num_devices=NCORES)
    xT_d = nc.dram_tensor("xT", [D, T], f32, kind="ExternalInput")
    x_d = nc.dram_tensor("x", [XPAD_ROWS, D], f32r, kind="ExternalInput")
    gwT_d = nc.dram_tensor("gwT", [D, E], f32, kind="ExternalInput")
    w1T_d = nc.dram_tensor("w1T", [D, I], f32r, kind="ExternalInput")
    w3T_d = nc.dram_tensor("w3T", [D, I], f32r, kind="ExternalInput")
    w2T_d = nc.dram_tensor("w2T", [I, D], f32r, kind="ExternalInput")
    utri_d = nc.dram_tensor("utri", [P, P], f32, kind="ExternalInput")
    ones_d = nc.dram_tensor("ones", [P, P], f32, kind="ExternalInput")
    ident_d = nc.dram_tensor("ident", [P, P], f32r, kind="ExternalInput")
    tidb_d = nc.dram_tensor("tidb", [P, E], f32, kind="ExternalInput")
    sr_d = nc.dram_tensor("sr", [P, CT * P], f32, kind="ExternalInput")
    y_d = nc.dram_tensor("y", [NQ * RSH, D], f32, kind="ExternalOutput")

    with tile.TileContext(nc) as tc:
        with tc.tile_pool(name="wpool", bufs=1) as wpool, \
             tc.tile_pool(name="xgpool", bufs=2) as xgpool, \
             tc.tile_pool(name="gpool", bufs=2) as gpool, \
             tc.tile_pool(name="wapool", bufs=5) as wapool, \
             tc.tile_pool(name="cpool", bufs=5) as cpool, \
             tc.tile_pool(name="xepool", bufs=3) as xepool, \
             tc.tile_pool(name="xtpool", bufs=1) as xtpool, \
             tc.tile_pool(name="apool", bufs=1) as apool, \
             tc.tile_pool(name="spool", bufs=2) as spool, \
             tc.tile_pool(name="ypool", bufs=2) as ypool, \
             tc.tile_pool(name="psum", bufs=2, space="PSUM") as psum, \
             tc.tile_pool(name="pyps", bufs=2, space="PSUM") as pyps, \
             tc.tile_pool(name="psmall", bufs=2, space="PSUM") as psmall, \
             tc.tile_pool(name="dram", bufs=1, space="DRAM") as dram:

            # --- constants + resident weights ---
            gwT_s = wpool.tile([P, DK, E], f32, tag="gw")
            nc.sync.dma_start(gwT_s[:], gwT_d[:, :].rearrange("(o p) e -> p o e", p=P))
            utri_s = wpool.tile([P, P], f32, tag="utri")
            nc.sync.dma_start(utri_s[:], utri_d[:, :])
            ones_s = wpool.tile([P, P], f32, tag="ones")
            nc.sync.dma_start(ones_s[:], ones_d[:, :])
            ident_s = wpool.tile([P, P], f32r, tag="ident")
            nc.sync.dma_start(ident_s[:], ident_d[:, :])
            tidb_s = wpool.tile([P, E], f32, tag="tidb")
            nc.sync.dma_start(tidb_s[:], tidb_d[:, :])
            sr_s = wpool.tile([P, CT * P], f32, tag="sr")
            nc.sync.dma_start(sr_s[:], sr_d[:, :])
            identf_s = wpool.tile([P, P], f32, tag="identf")
            nc.vector.tensor_copy(identf_s[:], ident_s[:])

            w1T_s = wpool.tile([P, DK, I], f32r, tag="w1")
            w3T_s = wpool.tile([P, DK, I], f32r, tag="w3")
            w2T_s = wpool.tile([P, IK, D], f32r, tag="w2")
            for h in range(4):
                hs = slice(h * (I // 4), (h + 1) * (I // 4))
                nc.scalar.dma_start(
                    w1T_s[:, :, hs], w1T_d[:, hs].rearrange("(o p) i -> p o i", p=P))
                nc.gpsimd.dma_start(
                    w3T_s[:, :, hs], w3T_d[:, hs].rearrange("(o p) i -> p o i", p=P))
                nc.scalar.dma_start(
                    w2T_s[:, :, hs], w2T_d[:, hs].rearrange("(o p) d -> p o d", p=P))

            ycontribs = [dram.tile([YC_ROWS, D], f32, tag=f"yc{q}", name=f"yc{q}")
                         for q in range(NQ)]
            yshards = [dram.tile([RSH, D], f32, tag=f"ys{q}", name=f"ys{q}")
                       for q in range(NQ)]

            # --- zero-fill contribution buffers & list pads (scalar queue:
            #     idle early, keeps sync free for input streaming) ---
            zt = wpool.tile([P, D], f32, tag="zt")
            nc.vector.memset(zt[:], 0.0)
            for q in range(NQ):
                for r in range(YC_ROWS // P):
                    nc.gpsimd.dma_start(ycontribs[q][r * P:(r + 1) * P, :], zt[:])

            # ============ phase A: gate for all ranges (true fp32) ============
            # scores^T [E, tokens] with N=512 matmuls, PE-transposed back to
            # [tokens, E] tiles for the softmax/top-2.
            wgt_alls = []
            for q in range(NQ):
                wgt_all = wapool.tile([P, E], f32, tag="wgtall", name=f"wa{q}")
                wgt_alls.append(wgt_all)
                for half in range(2):
                    t0 = q * RT + half * TCH
                    xg_s = xgpool.tile([P, DK, TCH], f32, tag="xg")
                    nc.sync.dma_start(
                        xg_s[:],
                        xT_d[:, t0:t0 + TCH].rearrange("(o p) t -> p o t", p=P))
                    ps_sT = psmall.tile([E, TCH], f32, tag="sm")
                    for dk in range(DK):
                        nc.tensor.matmul(
                            ps_sT[:], lhsT=gwT_s[:, dk, :], rhs=xg_s[:, dk, :],
                            start=(dk == 0), stop=(dk == DK - 1))
                    sT_sb = gpool.tile([E, TCH], f32, tag="sTsb")
                    nc.vector.tensor_copy(sT_sb[:], ps_sT[:])
                    for tt in range(4):
                        f = half * 4 + tt
                        ps_g = psmall.tile([P, E], f32, tag="sm")
                        nc.tensor.transpose(
                            ps_g[:], sT_sb[:, tt * P:(tt + 1) * P],
                            identf_s[:E, :E])
                        negmx = gpool.tile([P, 1], f32, tag="negmx")
                        nc.vector.tensor_reduce(
                            negmx[:], ps_g[:], mybir.AxisListType.X,
                            mybir.AluOpType.max)
                        nc.vector.tensor_scalar_mul(negmx[:], negmx[:], -1.0)
                        probs = gpool.tile([P, E], f32, tag="probs")
                        sumexp = gpool.tile([P, 1], f32, tag="sumexp")
                        nc.scalar.activation(
                            probs[:], ps_g[:], mybir.ActivationFunctionType.Exp,
                            bias=negmx[:, 0:1], accum_out=sumexp[:, 0:1])
                        recip = gpool.tile([P, 1], f32, tag="recip")
                        nc.vector.reciprocal(recip[:], sumexp[:])
                        nc.vector.tensor_scalar_mul(
                            probs[:], probs[:], recip[:, 0:1])
                        mx8 = gpool.tile([P, 8], f32, tag="mx8")
                        nc.vector.max(mx8[:], probs[:])
                        ge = gpool.tile([P, 1], f32, tag="ge")
                        nc.vector.tensor_tensor(
                            ge[:], probs[:, 0:1], mx8[:, 1:2],
                            mybir.AluOpType.is_ge)
                        nc.vector.tensor_mul(
                            wgt_all[:, f:f + 1], probs[:, 0:1], ge[:])

            # ===== phase B: compaction via prefix sums + one-hot matmuls =====
            # For each list slot s: gather-index/weight/occupancy recovered as
            # sum_t [pos[t]==s] * (tid, wgt, 1)[t]  -- no DRAM round trip.
            lists = []
            for q in range(NQ):
                wgt_all = wgt_alls[q]
                m = cpool.tile([P, E], f32, tag="m", name=f"m{q}")
                nc.vector.tensor_scalar(
                    m[:], wgt_all[:], 0.0, scalar2=None,
                    op0=mybir.AluOpType.is_gt)
                psA = psmall.tile([P, E], f32, tag="sm")
                nc.tensor.matmul(psA[:], lhsT=utri_s[:], rhs=m[:],
                                 start=True, stop=True)
                psC = psmall.tile([P, E], f32, tag="sm")
                nc.tensor.matmul(psC[:], lhsT=ones_s[:], rhs=m[:],
                                 start=True, stop=True)
                pos = cpool.tile([P, E], f32, tag="pos", name=f"pos{q}")
                nc.vector.tensor_copy(pos[:], psA[:])
                ctot = cpool.tile([P, E], f32, tag="ctot", name=f"ct{q}")
                nc.vector.tensor_copy(ctot[:], psC[:])
                for f in range(1, E):
                    nc.vector.tensor_add(
                        ctot[:, f:f + 1], ctot[:, f:f + 1], ctot[:, f - 1:f])
                for f in range(1, E):
                    nc.vector.tensor_add(
                        pos[:, f:f + 1], pos[:, f:f + 1], ctot[:, f - 1:f])
                nc.vector.tensor_scalar_add(pos[:], pos[:], float(-RT))
                nc.vector.tensor_mul(pos[:], pos[:], m[:])
                nc.vector.tensor_scalar_add(pos[:], pos[:], float(RT))

                # rhs payload per token: [tid, wgt, mask]
                pay = cpool.tile([P, E, 3], f32, tag="pay", name=f"pay{q}")
                nc.vector.tensor_scalar_add(
                    pay[:, :, 0], tidb_s[:], float(q * RT))
                nc.vector.tensor_copy(pay[:, :, 1], wgt_all[:])
                nc.vector.tensor_copy(pay[:, :, 2], m[:])

                lst = cpool.tile([P, CT, 3], f32, tag="lst", name=f"lst{q}")
                for ct in range(CT):
                    ps_l = psmall.tile([P, 3], f32, tag="sm")
                    for f in range(E):
                        ind = cpool.tile([P, P], f32, tag="ind")
                        nc.vector.tensor_tensor(
                            ind[:], pos[:, f:f + 1].to_broadcast([P, P]),
                            sr_s[:, ct * P:(ct + 1) * P],
                            mybir.AluOpType.is_equal)
                        nc.tensor.matmul(
                            ps_l[:], lhsT=ind[:], rhs=pay[:, f, :],
                            start=(f == 0), stop=(f == E - 1))
                    nc.vector.tensor_copy(lst[:, ct, :], ps_l[:])

                # pads (occ=0): gather trash x row, scatter to trash y row
                gidxf = cpool.tile([P, CT], f32, tag="gxf", name=f"gxf{q}")
                occ1 = cpool.tile([P, CT], f32, tag="occ1", name=f"occ1{q}")
                # gidx = tid + (1-occ)*T ; yidx = tid - q*RT + (1-occ)*(RT + q*RT)
                nc.vector.tensor_scalar(
                    occ1[:], lst[:, :, 2], -1.0, None,
                    op0=mybir.AluOpType.add)        # occ-1  (0 or -1)
                gidx_i = cpool.tile([P, CT], i32, tag="gidx", name=f"gi{q}")
                nc.vector.tensor_scalar(
                    gidxf[:], occ1[:], -float(T), None,
                    op0=mybir.AluOpType.mult)       # (1-occ)*T
                nc.vector.tensor_add(gidxf[:], gidxf[:], lst[:, :, 0])
                nc.vector.tensor_copy(gidx_i[:], gidxf[:])
                yidxf = cpool.tile([P, CT], f32, tag="yxf", name=f"yxf{q}")
                nc.vector.tensor_scalar(
                    yidxf[:], occ1[:], -float(RT + q * RT), None,
                    op0=mybir.AluOpType.mult)       # (1-occ)*(RT+q*RT)
                nc.vector.tensor_add(yidxf[:], yidxf[:], lst[:, :, 0])
                nc.vector.tensor_scalar_add(yidxf[:], yidxf[:], float(-q * RT))
                yidx_i = cpool.tile([P, CT], i32, tag="yidxi", name=f"yi{q}")
                nc.vector.tensor_copy(yidx_i[:], yidxf[:])
                lists.append((lst, gidx_i, yidx_i))

            # ============ phase C: per-range gather/compute/combine ============
            for q in range(NQ):
                lst, gidx, yidxi = lists[q]
                xeT = xtpool.tile([P, DK, CAP], f32r, tag="xeT")
                for ct in range(CT):
                    xe = xepool.tile([P, D], f32r, tag="xe")
                    nc.gpsimd.indirect_dma_start(
                        out=xe[:],
                        out_offset=None,
                        in_=x_d[:, :],
                        in_offset=bass.IndirectOffsetOnAxis(
                            ap=gidx[:, ct:ct + 1], axis=0))
                    for dk in range(DK):
                        ptr = psmall.tile([P, P], f32r, tag="sm")
                        nc.tensor.transpose(
                            ptr[:], xe[:, dk * P:(dk + 1) * P], ident_s[:])
                        nc.vector.tensor_copy(
                            xeT[:, dk, ct * P:(ct + 1) * P], ptr[:])

                aT = apool.tile([P, IK, CAP], f32r, tag="aT")
                for ik in range(IK):
                    isl = slice(ik * P, (ik + 1) * P)
                    ph = psum.tile([P, CAP], f32, tag="ph")
                    for dk in range(DK):
                        nc.tensor.matmul(
                            ph[:], lhsT=w1T_s[:, dk, isl], rhs=xeT[:, dk, :],
                            start=(dk == 0), stop=(dk == DK - 1))
                    pg = psum.tile([P, CAP], f32, tag="pg")
                    for dk in range(DK):
                        nc.tensor.matmul(
                            pg[:], lhsT=w3T_s[:, dk, isl], rhs=xeT[:, dk, :],
                            start=(dk == 0), stop=(dk == DK - 1))
                    sil = spool.tile([P, CAP], f32r, tag="sil")
                    nc.scalar.activation(
                        sil[:], ph[:], mybir.ActivationFunctionType.Silu)
                    nc.vector.tensor_mul(aT[:, ik, :], sil[:], pg[:])

                for ct in range(CT):
                    yt = ypool.tile([P, D], f32, tag="yt")
                    for dc in range(2):
                        py = pyps.tile([P, TCH], f32, tag="py")
                        for ik in range(IK):
                            nc.tensor.matmul(
                                py[:],
                                lhsT=aT[:, ik, ct * P:(ct + 1) * P],
                                rhs=w2T_s[:, ik, dc * TCH:(dc + 1) * TCH],
                                start=(ik == 0), stop=(ik == IK - 1))
                        nc.vector.tensor_scalar_mul(
                            yt[:, dc * TCH:(dc + 1) * TCH], py[:],
                            lst[:, ct, 1:2])
                    nc.gpsimd.indirect_dma_start(
                        out=ycontribs[q][:, :],
                        out_offset=bass.IndirectOffsetOnAxis(
                            ap=yidxi[:, ct:ct + 1], axis=0),
                        in_=yt[:],
                        in_offset=None)

                nc.gpsimd.collective_compute(
                    "ReduceScatter",
                    mybir.AluOpType.add,
                    replica_groups=[list(range(NCORES))],
                    ins=[ycontribs[q][0:RT, :].opt()],
                    outs=[yshards[q].opt()],
                )

            # ============ phase D: ship shards to the output ============
            for q in range(NQ):
                nc.sync.dma_start(y_d[q * RSH:(q + 1) * RSH, :], yshards[q][:])
    nc.compile()
    return nc


def _get_nc():
    global _CACHED_NC
    if _CACHED_NC is None:
        _CACHED_NC = _build()
    return _CACHED_NC


def _in_maps(x, gate_w, w1, w3, w2):
    x = np.asarray(x, dtype=np.float32)
    gate_w = np.asarray(gate_w, dtype=np.float32)
    xT = np.ascontiguousarray(x.T)
    xpad = np.zeros((XPAD_ROWS, D), dtype=np.float32)
    xpad[:T] = x

    # host-side capacity check against the actual gate (cheap, exact)
    s = x @ gate_w.T
    thr = np.sort(s, axis=1)[:, -TOPK]          # 2nd-largest score
    routed = s >= thr[:, None]                  # [T, E]
    cnt = routed.reshape(NQ, RT, E).sum(axis=1)  # [NQ, E]
    if cnt.max() > CAP:
        raise RuntimeError(f"routing capacity exceeded: {cnt.max()} > {CAP}")

    utri = np.triu(np.ones((P, P), np.float32), k=1)
    ones = np.ones((P, P), np.float32)
    ident = np.eye(P, dtype=np.float32)
    tidb = (np.arange(E)[None, :] * P + np.arange(P)[:, None]).astype(np.float32)
    sr = np.broadcast_to(np.arange(CT * P, dtype=np.float32)[None, :],
                         (P, CT * P)).copy()

    maps = []
    for e in range(NCORES):
        perm = [e] + [j for j in range(E) if j != e]
        gwT = np.ascontiguousarray(gate_w[perm].T)
        maps.append({
            "xT": xT,
            "x": xpad,
            "gwT": gwT,
            "w1T": np.ascontiguousarray(np.asarray(w1[e], np.float32).T),
            "w3T": np.ascontiguousarray(np.asarray(w3[e], np.float32).T),
            "w2T": np.ascontiguousarray(np.asarray(w2[e], np.float32).T),
            "utri": utri,
            "ones": ones,
            "ident": ident,
            "tidb": tidb,
            "sr": sr,
        })
    return maps


def run(x, gate_w, w1, w3, w2, trace=False, trace_cores=None):
    nc = _get_nc()
    maps = _in_maps(x, gate_w, w1, w3, w2)
    res = run_bass_kernel_spmd(
        nc, maps, core_ids=list(range(NCORES)), trace=trace,
        trace_cores=trace_cores)
    # core r's output block q (128 rows) holds tokens [1024q + 128r, +128)
    y = np.empty((T, D), dtype=np.float32)
    for r in range(NCORES):
        yr = res.results[r]["y"]
        for q in range(NQ):
            t0 = q * RT + r * RSH
            y[t0:t0 + RSH] = yr[q * RSH:(q + 1) * RSH]
    return y, res


def kernel(x, gate_w, w1, w3, w2):
    y, _ = run(x, gate_w, w1, w3, w2, trace=False)
    return y.astype(np.float32)



# revision 3
# speedup vs baseline: 1.1387x; 1.1387x over previous
"""MoE SwiGLU (T=4096, D=I=1024, E=8, top-2) on 8 Trainium2 NeuronCores.

Expert-parallel with on-device routing: core e holds expert e's weights
(bf16) in SBUF.  The gate (scores -> softmax -> top-2) is replicated on
every core: matmuls in f32r (single PE pass), softmax math in fp32 on
raw exp values (logits are O(1), no max-shift needed; top-2 selection
on exp is monotone-equivalent).  Each core COMPACTS the token ids
routed to its expert (matmul prefix-sums + one-hot matmuls with exact
small-integer payload in bf16), gathers just those x rows (indirect
DMA), computes SwiGLU in bf16 at full PE rate, scales by the routing
weight, and scatters bf16 rows into a per-range contribution buffer
that was lazily zero-filled.  Four token-range bf16 ReduceScatters
overlap compute; shards are cast back to fp32 on-chip.

The whole thing is software-pipelined per 1024-token range: the gate
for range q+1 is issued between the compaction and the SwiGLU of range
q so the vector-engine softmax hides under the tensor-engine matmul
stream.
"""
import os
import sys

import numpy as np

for _p in ("/opt/trn_rl_repo", "/root/.axon_site/_ro/trn_rl_repo"):
    if os.path.isdir(_p) and _p not in sys.path:
        sys.path.append(_p)

import concourse.bass as bass  # noqa: E402
import concourse.mybir as mybir  # noqa: E402
import concourse.tile as tile  # noqa: E402
from concourse import bacc  # noqa: E402
from concourse.bass_utils import run_bass_kernel_spmd  # noqa: E402

P = 128
T, D, I, E, TOPK = 4096, 1024, 1024, 8, 2
NCORES = 8
TCH = 512            # gate token chunk (matmul free dim)
DK = D // P          # 8
IK = I // P          # 8
NQ = 4               # ReduceScatter ranges
RT = T // NQ         # 1024 tokens per range
NF = RT // P         # 8 token f-tiles per range
RSH = RT // NCORES   # 128-token shard per core per range
CAP = 384            # routed-token capacity per (core, range)
CT = CAP // P        # 3 c-tiles per range
YC_ROWS = RT + P     # contribution rows + trash row region
XPAD_ROWS = T + P    # x padded with zero rows (gather trash target)
f32 = mybir.dt.float32
f32r = mybir.dt.float32r
bf16 = mybir.dt.bfloat16
i32 = mybir.dt.int32

_CACHED_NC = None


def _build():
    nc = bacc.Bacc("TRN2", target_bir_lowering=False, debug=False,
                   num_devices=NCORES)
    xT_d = nc.dram_tensor("xT", [D, T], f32r, kind="ExternalInput")
    x_d = nc.dram_tensor("x", [XPAD_ROWS, D], f32r, kind="ExternalInput")
    gwT_d = nc.dram_tensor("gwT", [D, E], f32r, kind="ExternalInput")
    w1T_d = nc.dram_tensor("w1T", [D, I], bf16, kind="ExternalInput")
    w3T_d = nc.dram_tensor("w3T", [D, I], bf16, kind="ExternalInput")
    w2T_d = nc.dram_tensor("w2T", [I, D], bf16, kind="ExternalInput")
    utri_d = nc.dram_tensor("utri", [P, P], f32, kind="ExternalInput")
    ones_d = nc.dram_tensor("ones", [P, P], f32, kind="ExternalInput")
    ident_d = nc.dram_tensor("ident", [P, P], f32r, kind="ExternalInput")
    pcol_d = nc.dram_tensor("pcol", [P, NF], bf16, kind="ExternalInput")
    fcol_d = nc.dram_tensor("fcol", [P, NF], bf16, kind="ExternalInput")
    sr_d = nc.dram_tensor("sr", [P, CT * P], f32, kind="ExternalInput")
    y_d = nc.dram_tensor("y", [NQ * RSH, D], f32, kind="ExternalOutput")

    with tile.TileContext(nc) as tc:
        with tc.tile_pool(name="wpool", bufs=1) as wpool, \
             tc.tile_pool(name="xgpool", bufs=3) as xgpool, \
             tc.tile_pool(name="gpool", bufs=2) as gpool, \
             tc.tile_pool(name="wapool", bufs=2) as wapool, \
             tc.tile_pool(name="cpool", bufs=2) as cpool, \
             tc.tile_pool(name="xepool", bufs=4) as xepool, \
             tc.tile_pool(name="xtpool", bufs=2) as xtpool, \
             tc.tile_pool(name="apool", bufs=2) as apool, \
             tc.tile_pool(name="spool", bufs=2) as spool, \
             tc.tile_pool(name="ypool", bufs=2) as ypool, \
             tc.tile_pool(name="psum", bufs=2, space="PSUM") as psum, \
             tc.tile_pool(name="pyps", bufs=2, space="PSUM") as pyps, \
             tc.tile_pool(name="psmall", bufs=2, space="PSUM") as psmall, \
             tc.tile_pool(name="dram", bufs=1, space="DRAM") as dram:

            # --- constants + resident weights ---
            gwT_s = wpool.tile([P, DK, E], f32r, tag="gw")
            nc.sync.dma_start(gwT_s[:], gwT_d[:, :].rearrange("(o p) e -> p o e", p=P))
            utri_s = wpool.tile([P, P], f32, tag="utri")
            nc.sync.dma_start(utri_s[:], utri_d[:, :])
            ones_s = wpool.tile([P, P], f32, tag="ones")
            nc.sync.dma_start(ones_s[:], ones_d[:, :])
            ident_s = wpool.tile([P, P], f32r, tag="ident")
            nc.sync.dma_start(ident_s[:], ident_d[:, :])
            pcol_s = wpool.tile([P, NF], bf16, tag="pcol")
            nc.sync.dma_start(pcol_s[:], pcol_d[:, :])
            fcol_s = wpool.tile([P, NF], bf16, tag="fcol")
            nc.sync.dma_start(fcol_s[:], fcol_d[:, :])
            sr_s = wpool.tile([P, CT * P], f32, tag="sr")
            nc.sync.dma_start(sr_s[:], sr_d[:, :])

            w1T_s = wpool.tile([P, DK, I], bf16, tag="w1")
            w3T_s = wpool.tile([P, DK, I], bf16, tag="w3")
            w2T_s = wpool.tile([P, IK, D], bf16, tag="w2")
            for h in range(4):
                hs = slice(h * (I // 4), (h + 1) * (I // 4))
                nc.scalar.dma_start(
                    w1T_s[:, :, hs], w1T_d[:, hs].rearrange("(o p) i -> p o i", p=P))
                nc.gpsimd.dma_start(
                    w3T_s[:, :, hs], w3T_d[:, hs].rearrange("(o p) i -> p o i", p=P))
                nc.scalar.dma_start(
                    w2T_s[:, :, hs], w2T_d[:, hs].rearrange("(o p) d -> p o d", p=P))

            ycontribs = [dram.tile([YC_ROWS, D], bf16, tag=f"yc{q}", name=f"yc{q}")
                         for q in range(NQ)]
            yshards = [dram.tile([RSH, D], bf16, tag=f"ys{q}", name=f"ys{q}")
                       for q in range(NQ)]

            zt = wpool.tile([P, D], bf16, tag="zt")
            nc.vector.memset(zt[:], 0.0)

            def zerofill(q):
                for r in range(YC_ROWS // P):
                    nc.gpsimd.dma_start(ycontribs[q][r * P:(r + 1) * P, :], zt[:])

            # --- gate for one range: f32r matmuls, fp32 softmax on raw exp ---
            def gate(q):
                wgt_all = wapool.tile([P, NF], f32, tag="wgtall", name=f"wa{q}")
                for half in range(2):
                    t0 = q * RT + half * TCH
                    xg_s = xgpool.tile([P, DK, TCH], f32r, tag="xg")
                    nc.sync.dma_start(
                        xg_s[:],
                        xT_d[:, t0:t0 + TCH].rearrange("(o p) t -> p o t", p=P))
                    ps_sT = psmall.tile([E, TCH], f32, tag="sm")
                    for dk in range(DK):
                        nc.tensor.matmul(
                            ps_sT[:], lhsT=gwT_s[:, dk, :], rhs=xg_s[:, dk, :],
                            start=(dk == 0), stop=(dk == DK - 1))
                    sT_sb = gpool.tile([E, TCH], f32r, tag="sTsb")
                    nc.vector.tensor_copy(sT_sb[:], ps_sT[:])
                    for tt in range(4):
                        f = half * 4 + tt
                        ps_g = psmall.tile([P, E], f32r, tag="sm")
                        nc.tensor.transpose(
                            ps_g[:], sT_sb[:, tt * P:(tt + 1) * P],
                            ident_s[:E, :E])
                        exps = gpool.tile([P, E], f32, tag="exps")
                        sumexp = gpool.tile([P, 1], f32, tag="sumexp")
                        nc.scalar.activation(
                            exps[:], ps_g[:], mybir.ActivationFunctionType.Exp,
                            accum_out=sumexp[:, 0:1])
                        mx8 = gpool.tile([P, 8], f32, tag="mx8")
                        nc.vector.max(mx8[:], exps[:])
                        ge = gpool.tile([P, 1], f32, tag="ge")
                        nc.vector.tensor_tensor(
                            ge[:], exps[:, 0:1], mx8[:, 1:2],
                            mybir.AluOpType.is_ge)
                        recip = gpool.tile([P, 1], f32, tag="recip")
                        nc.vector.reciprocal(recip[:], sumexp[:])
                        w0 = gpool.tile([P, 1], f32, tag="w0")
                        nc.vector.tensor_mul(w0[:], exps[:, 0:1], recip[:])
                        nc.vector.tensor_mul(
                            wgt_all[:, f:f + 1], w0[:], ge[:])
                return wgt_all

            # --- compaction: prefix sums + one-hot matmuls (bf16 payload) ---
            def compact(q, wgt_all):
                m = cpool.tile([P, NF], f32, tag="m", name=f"m{q}")
                nc.vector.tensor_scalar(
                    m[:], wgt_all[:], 0.0, scalar2=None,
                    op0=mybir.AluOpType.is_gt)
                psA = psmall.tile([P, NF], f32, tag="sm")
                nc.tensor.matmul(psA[:], lhsT=utri_s[:], rhs=m[:],
                                 start=True, stop=True)
                psC = psmall.tile([P, NF], f32, tag="sm")
                nc.tensor.matmul(psC[:], lhsT=ones_s[:], rhs=m[:],
                                 start=True, stop=True)
                pos = cpool.tile([P, NF], f32, tag="pos", name=f"pos{q}")
                nc.vector.tensor_copy(pos[:], psA[:])
                ctot = cpool.tile([P, NF], f32, tag="ctot", name=f"ct{q}")
                nc.vector.tensor_copy(ctot[:], psC[:])
                for f in range(1, NF):
                    nc.vector.tensor_add(
                        ctot[:, f:f + 1], ctot[:, f:f + 1], ctot[:, f - 1:f])
                for f in range(1, NF):
                    nc.vector.tensor_add(
                        pos[:, f:f + 1], pos[:, f:f + 1], ctot[:, f - 1:f])
                # pads: pos -> RT (outside sr range) so no one-hot matches
                nc.vector.tensor_scalar_add(pos[:], pos[:], float(-RT))
                nc.vector.tensor_mul(pos[:], pos[:], m[:])
                nc.vector.tensor_scalar_add(pos[:], pos[:], float(RT))

                # rhs payload per token: [p, f, wgt, mask] (p,f exact in bf16)
                pay = cpool.tile([P, NF, 4], bf16, tag="pay", name=f"pay{q}")
                nc.vector.tensor_copy(pay[:, :, 0], pcol_s[:])
                nc.vector.tensor_copy(pay[:, :, 1], fcol_s[:])
                nc.vector.tensor_copy(pay[:, :, 2], wgt_all[:])
                nc.vector.tensor_copy(pay[:, :, 3], m[:])

                lstf = cpool.tile([P, CT, 4], f32, tag="lst", name=f"lst{q}")
                for ct in range(CT):
                    ps_l = psmall.tile([P, 4], f32, tag="sm")
                    for f in range(NF):
                        ind = cpool.tile([P, P], bf16, tag="ind")
                        nc.vector.tensor_tensor(
                            ind[:], pos[:, f:f + 1].to_broadcast([P, P]),
                            sr_s[:, ct * P:(ct + 1) * P],
                            mybir.AluOpType.is_equal)
                        nc.tensor.matmul(
                            ps_l[:], lhsT=ind[:], rhs=pay[:, f, :],
                            start=(f == 0), stop=(f == NF - 1))
                    nc.vector.tensor_copy(lstf[:, ct, :], ps_l[:])

                # recover indices; pads (occ=0): gather trash x row, scatter
                # to trash y row.  base = f*128 + p  (pads -> 0)
                base = cpool.tile([P, CT], f32, tag="base", name=f"b{q}")
                nc.vector.tensor_scalar(
                    base[:], lstf[:, :, 1], 128.0, scalar2=None,
                    op0=mybir.AluOpType.mult)
                nc.vector.tensor_add(base[:], base[:], lstf[:, :, 0])
                occ1 = cpool.tile([P, CT], f32, tag="occ1", name=f"o{q}")
                nc.vector.tensor_scalar(
                    occ1[:], lstf[:, :, 3], -1.0, scalar2=None,
                    op0=mybir.AluOpType.add)        # occ-1  (0 or -1)
                # gidx = base + q*RT + (1-occ)*(T - q*RT)
                gidxf = cpool.tile([P, CT], f32, tag="gxf", name=f"gxf{q}")
                nc.vector.tensor_scalar(
                    gidxf[:], occ1[:], -float(T - q * RT), scalar2=None,
                    op0=mybir.AluOpType.mult)
                nc.vector.tensor_add(gidxf[:], gidxf[:], base[:])
                nc.vector.tensor_scalar_add(gidxf[:], gidxf[:], float(q * RT))
                gidx_i = cpool.tile([P, CT], i32, tag="gidx", name=f"gi{q}")
                nc.vector.tensor_copy(gidx_i[:], gidxf[:])
                # yidx = base + (1-occ)*RT
                yidxf = cpool.tile([P, CT], f32, tag="yxf", name=f"yxf{q}")
                nc.vector.tensor_scalar(
                    yidxf[:], occ1[:], -float(RT), scalar2=None,
                    op0=mybir.AluOpType.mult)
                nc.vector.tensor_add(yidxf[:], yidxf[:], base[:])
                yidx_i = cpool.tile([P, CT], i32, tag="yidxi", name=f"yi{q}")
                nc.vector.tensor_copy(yidx_i[:], yidxf[:])
                return lstf, gidx_i, yidx_i

            # ---------------- the per-range pipeline ----------------
            zerofill(0)
            zerofill(1)
            wgt_next = gate(0)
            for q in range(NQ):
                lstf, gidx, yidxi = compact(q, wgt_next)
                if q + 1 < NQ:
                    wgt_next = gate(q + 1)

                # gather + PE-transpose (+cast to bf16)
                xeT = xtpool.tile([P, DK, CAP], bf16, tag="xeT")
                for ct in range(CT):
                    xe = xepool.tile([P, D], f32r, tag="xe")
                    nc.gpsimd.indirect_dma_start(
                        out=xe[:],
                        out_offset=None,
                        in_=x_d[:, :],
                        in_offset=bass.IndirectOffsetOnAxis(
                            ap=gidx[:, ct:ct + 1], axis=0))
                    for dk in range(DK):
                        ptr = psmall.tile([P, P], f32r, tag="sm")
                        nc.tensor.transpose(
                            ptr[:], xe[:, dk * P:(dk + 1) * P], ident_s[:])
                        nc.vector.tensor_copy(
                            xeT[:, dk, ct * P:(ct + 1) * P], ptr[:])

                # SwiGLU in bf16
                aT = apool.tile([P, IK, CAP], bf16, tag="aT")
                for ik in range(IK):
                    isl = slice(ik * P, (ik + 1) * P)
                    ph = psum.tile([P, CAP], f32, tag="ph")
                    for dk in range(DK):
                        nc.tensor.matmul(
                            ph[:], lhsT=w1T_s[:, dk, isl], rhs=xeT[:, dk, :],
                            start=(dk == 0), stop=(dk == DK - 1))
                    pg = psum.tile([P, CAP], f32, tag="pg")
                    for dk in range(DK):
                        nc.tensor.matmul(
                            pg[:], lhsT=w3T_s[:, dk, isl], rhs=xeT[:, dk, :],
                            start=(dk == 0), stop=(dk == DK - 1))
                    sil = spool.tile([P, CAP], f32r, tag="sil")
                    nc.scalar.activation(
                        sil[:], ph[:], mybir.ActivationFunctionType.Silu)
                    nc.vector.tensor_mul(aT[:, ik, :], sil[:], pg[:])

                # w2 + routing-weight scale + scatter (bf16 rows)
                for ct in range(CT):
                    yt = ypool.tile([P, D], bf16, tag="yt")
                    for dc in range(2):
                        py = pyps.tile([P, TCH], f32, tag="py")
                        for ik in range(IK):
                            nc.tensor.matmul(
                                py[:],
                                lhsT=aT[:, ik, ct * P:(ct + 1) * P],
                                rhs=w2T_s[:, ik, dc * TCH:(dc + 1) * TCH],
                                start=(ik == 0), stop=(ik == IK - 1))
                        nc.vector.tensor_scalar_mul(
                            yt[:, dc * TCH:(dc + 1) * TCH], py[:],
                            lstf[:, ct, 2:3])
                    nc.gpsimd.indirect_dma_start(
                        out=ycontribs[q][:, :],
                        out_offset=bass.IndirectOffsetOnAxis(
                            ap=yidxi[:, ct:ct + 1], axis=0),
                        in_=yt[:],
                        in_offset=None)

                if q + 2 < NQ:
                    zerofill(q + 2)

                nc.gpsimd.collective_compute(
                    "ReduceScatter",
                    mybir.AluOpType.add,
                    replica_groups=[list(range(NCORES))],
                    ins=[ycontribs[q][0:RT, :].opt()],
                    outs=[yshards[q].opt()],
                )

            # ------------- ship shards to the fp32 output -------------
            for q in range(NQ):
                ysb = ypool.tile([P, D], bf16, tag="ysb")
                nc.sync.dma_start(ysb[:], yshards[q][:])
                yf = ypool.tile([P, D], f32, tag="yf")
                nc.vector.tensor_copy(yf[:], ysb[:])
                nc.sync.dma_start(y_d[q * RSH:(q + 1) * RSH, :], yf[:])
    nc.compile()
    return nc


def _get_nc():
    global _CACHED_NC
    if _CACHED_NC is None:
        _CACHED_NC = _build()
    return _CACHED_NC


def _in_maps(x, gate_w, w1, w3, w2):
    import ml_dtypes
    bf = ml_dtypes.bfloat16
    x = np.asarray(x, dtype=np.float32)
    gate_w = np.asarray(gate_w, dtype=np.float32)
    xT = np.ascontiguousarray(x.T)
    xpad = np.zeros((XPAD_ROWS, D), dtype=np.float32)
    xpad[:T] = x

    # host-side capacity check against the actual gate (cheap, exact)
    s = x @ gate_w.T
    thr = np.sort(s, axis=1)[:, -TOPK]          # 2nd-largest score
    routed = s >= thr[:, None]                  # [T, E]
    cnt = routed.reshape(NQ, RT, E).sum(axis=1)  # [NQ, E]
    if cnt.max() > CAP:
        raise RuntimeError(f"routing capacity exceeded: {cnt.max()} > {CAP}")

    utri = np.triu(np.ones((P, P), np.float32), k=1)
    ones = np.ones((P, P), np.float32)
    ident = np.eye(P, dtype=np.float32)
    pcol = np.broadcast_to(np.arange(P, dtype=np.float32)[:, None],
                           (P, NF)).astype(bf)
    fcol = np.broadcast_to(np.arange(NF, dtype=np.float32)[None, :],
                           (P, NF)).astype(bf)
    sr = np.broadcast_to(np.arange(CT * P, dtype=np.float32)[None, :],
                         (P, CT * P)).copy()

    maps = []
    for e in range(NCORES):
        perm = [e] + [j for j in range(E) if j != e]
        gwT = np.ascontiguousarray(gate_w[perm].T)
        maps.append({
            "xT": xT,
            "x": xpad,
            "gwT": gwT,
            "w1T": np.ascontiguousarray(np.asarray(w1[e], np.float32).T).astype(bf),
            "w3T": np.ascontiguousarray(np.asarray(w3[e], np.float32).T).astype(bf),
            "w2T": np.ascontiguousarray(np.asarray(w2[e], np.float32).T).astype(bf),
            "utri": utri,
            "ones": ones,
            "ident": ident,
            "pcol": pcol,
            "fcol": fcol,
            "sr": sr,
        })
    return maps


def run(x, gate_w, w1, w3, w2, trace=False, trace_cores=None):
    nc = _get_nc()
    maps = _in_maps(x, gate_w, w1, w3, w2)
    res = run_bass_kernel_spmd(
        nc, maps, core_ids=list(range(NCORES)), trace=trace,
        trace_cores=trace_cores)
    # core r's output block q (128 rows) holds tokens [1024q + 128r, +128)
    y = np.empty((T, D), dtype=np.float32)
    for r in range(NCORES):
        yr = res.results[r]["y"]
        for q in range(NQ):
            t0 = q * RT + r * RSH
            y[t0:t0 + RSH] = yr[q * RSH:(q + 1) * RSH]
    return y, res


def kernel(x, gate_w, w1, w3, w2):
    y, _ = run(x, gate_w, w1, w3, w2, trace=False)
    return y.astype(np.float32)


# revision 5
# speedup vs baseline: 1.2225x; 1.0736x over previous
"""MoE SwiGLU (T=4096, D=I=1024, E=8, top-2) on 8 Trainium2 NeuronCores.

Expert-parallel with on-device routing: core e holds expert e's weights
(bf16) in SBUF.  The gate (scores -> softmax -> top-2) is replicated on
every core: matmuls in f32r (single PE pass), softmax math in fp32 on
raw exp values (logits are O(1), no max-shift needed; top-2 selection
on exp is monotone-equivalent).  Each core COMPACTS the token ids
routed to its expert (matmul prefix-sums + one-hot matmuls with exact
small-integer payload in bf16), gathers just those x rows (indirect
DMA), computes SwiGLU in bf16 at full PE rate, scales by the routing
weight, and scatters bf16 rows into a per-range contribution buffer
that was lazily zero-filled.  Four token-range bf16 ReduceScatters
overlap compute; shards are cast back to fp32 on-chip.

Scheduling notes (learned from traces):
- collective_compute and indirect DMA are both gpsimd-queue-only, and a
  collective occupies the queue until it completes.  So range q+1's
  gathers are issued BEFORE RS(q) on that queue.
- the scalar queue carries only activations (Exp/Silu); weight loads
  ride sync/gpsimd as three full-row DMAs (2 KiB descriptor lines).
- the gate for range q+1 is issued between the transposes and SwiGLU of
  range q so the vector softmax hides under the PE matmul stream.
"""
import os
import sys

import numpy as np

for _p in ("/opt/trn_rl_repo", "/root/.axon_site/_ro/trn_rl_repo"):
    if os.path.isdir(_p) and _p not in sys.path:
        sys.path.append(_p)

import concourse.bass as bass  # noqa: E402
import concourse.mybir as mybir  # noqa: E402
import concourse.tile as tile  # noqa: E402
from concourse import bacc  # noqa: E402
from concourse.bass_utils import run_bass_kernel_spmd  # noqa: E402

P = 128
T, D, I, E, TOPK = 4096, 1024, 1024, 8, 2
NCORES = 8
TCH = 512            # gate token chunk (matmul free dim)
DK = D // P          # 8
IK = I // P          # 8
NQ = 4               # ReduceScatter ranges
RT = T // NQ         # 1024 tokens per range
NF = RT // P         # 8 token f-tiles per range
RSH = RT // NCORES   # 128-token shard per core per range
CAP = 320            # routed-token capacity per (core, range); actual max 281
CTS = [128, 128, 64]  # c-tile heights (sum = CAP)
CT = len(CTS)
YC_ROWS = RT + P     # contribution rows + trash row region
XPAD_ROWS = T + P    # x padded with zero rows (gather trash target)
f32 = mybir.dt.float32
f32r = mybir.dt.float32r
bf16 = mybir.dt.bfloat16
i32 = mybir.dt.int32

_CACHED_NC = None


def _build():
    nc = bacc.Bacc("TRN2", target_bir_lowering=False, debug=False,
                   num_devices=NCORES)
    xT_d = nc.dram_tensor("xT", [D, T], f32r, kind="ExternalInput")
    x_d = nc.dram_tensor("x", [XPAD_ROWS, D], f32r, kind="ExternalInput")
    gwT_d = nc.dram_tensor("gwT", [D, E], f32r, kind="ExternalInput")
    w1T_d = nc.dram_tensor("w1T", [D, I], bf16, kind="ExternalInput")
    w3T_d = nc.dram_tensor("w3T", [D, I], bf16, kind="ExternalInput")
    w2T_d = nc.dram_tensor("w2T", [I, D], bf16, kind="ExternalInput")
    utri_d = nc.dram_tensor("utri", [P, P], f32, kind="ExternalInput")
    ones_d = nc.dram_tensor("ones", [P, P], f32, kind="ExternalInput")
    ident_d = nc.dram_tensor("ident", [P, P], f32r, kind="ExternalInput")
    pcol_d = nc.dram_tensor("pcol", [P, NF], bf16, kind="ExternalInput")
    fcol_d = nc.dram_tensor("fcol", [P, NF], bf16, kind="ExternalInput")
    sr_d = nc.dram_tensor("sr", [P, CAP], f32, kind="ExternalInput")
    y_d = nc.dram_tensor("y", [NQ * RSH, D], f32, kind="ExternalOutput")

    with tile.TileContext(nc) as tc:
        with tc.tile_pool(name="wpool", bufs=1) as wpool, \
             tc.tile_pool(name="xgpool", bufs=3) as xgpool, \
             tc.tile_pool(name="gpool", bufs=2) as gpool, \
             tc.tile_pool(name="wapool", bufs=2) as wapool, \
             tc.tile_pool(name="cpool", bufs=2) as cpool, \
             tc.tile_pool(name="xepool", bufs=6) as xepool, \
             tc.tile_pool(name="xtpool", bufs=2) as xtpool, \
             tc.tile_pool(name="apool", bufs=2) as apool, \
             tc.tile_pool(name="spool", bufs=2) as spool, \
             tc.tile_pool(name="ypool", bufs=2) as ypool, \
             tc.tile_pool(name="psum", bufs=2, space="PSUM") as psum, \
             tc.tile_pool(name="pyps", bufs=2, space="PSUM") as pyps, \
             tc.tile_pool(name="psmall", bufs=2, space="PSUM") as psmall, \
             tc.tile_pool(name="dram", bufs=1, space="DRAM") as dram:

            # --- gate weights first (gate(0) needs them immediately) ---
            gwT_s = wpool.tile([P, DK, E], f32r, tag="gw")
            nc.sync.dma_start(gwT_s[:], gwT_d[:, :].rearrange("(o p) e -> p o e", p=P))

            ycontribs = [dram.tile([YC_ROWS, D], bf16, tag=f"yc{q}", name=f"yc{q}")
                         for q in range(NQ)]
            yshards = [dram.tile([RSH, D], bf16, tag=f"ys{q}", name=f"ys{q}")
                       for q in range(NQ)]

            # --- gate for one range: f32r matmuls, fp32 softmax on raw exp ---
            def gate(q):
                wgt_all = wapool.tile([P, NF], f32, tag="wgtall", name=f"wa{q}")
                for half in range(2):
                    t0 = q * RT + half * TCH
                    xg_s = xgpool.tile([P, DK, TCH], f32r, tag="xg")
                    nc.sync.dma_start(
                        xg_s[:],
                        xT_d[:, t0:t0 + TCH].rearrange("(o p) t -> p o t", p=P))
                    ps_sT = psmall.tile([E, TCH], f32, tag="sm")
                    for dk in range(DK):
                        nc.tensor.matmul(
                            ps_sT[:], lhsT=gwT_s[:, dk, :], rhs=xg_s[:, dk, :],
                            start=(dk == 0), stop=(dk == DK - 1))
                    sT_sb = gpool.tile([E, TCH], f32r, tag="sTsb")
                    nc.vector.tensor_copy(sT_sb[:], ps_sT[:])
                    for tt in range(4):
                        f = half * 4 + tt
                        ps_g = psmall.tile([P, E], f32r, tag="sm")
                        nc.tensor.transpose(
                            ps_g[:], sT_sb[:, tt * P:(tt + 1) * P],
                            ident_s[:E, :E])
                        exps = gpool.tile([P, E], f32, tag="exps")
                        sumexp = gpool.tile([P, 1], f32, tag="sumexp")
                        nc.scalar.activation(
                            exps[:], ps_g[:], mybir.ActivationFunctionType.Exp,
                            accum_out=sumexp[:, 0:1])
                        mx8 = gpool.tile([P, 8], f32, tag="mx8")
                        nc.vector.max(mx8[:], exps[:])
                        ge = gpool.tile([P, 1], f32, tag="ge")
                        nc.vector.tensor_tensor(
                            ge[:], exps[:, 0:1], mx8[:, 1:2],
                            mybir.AluOpType.is_ge)
                        recip = gpool.tile([P, 1], f32, tag="recip")
                        nc.vector.reciprocal(recip[:], sumexp[:])
                        w0 = gpool.tile([P, 1], f32, tag="w0")
                        nc.vector.tensor_mul(w0[:], exps[:, 0:1], recip[:])
                        nc.vector.tensor_mul(
                            wgt_all[:, f:f + 1], w0[:], ge[:])
                return wgt_all

            # --- compaction: prefix sums + one-hot matmuls (bf16 payload) ---
            def compact(q, wgt_all):
                m = cpool.tile([P, NF], f32, tag="m", name=f"m{q}")
                nc.vector.tensor_scalar(
                    m[:], wgt_all[:], 0.0, scalar2=None,
                    op0=mybir.AluOpType.is_gt)
                psA = psmall.tile([P, NF], f32, tag="sm")
                nc.tensor.matmul(psA[:], lhsT=utri_s[:], rhs=m[:],
                                 start=True, stop=True)
                psC = psmall.tile([P, NF], f32, tag="sm")
                nc.tensor.matmul(psC[:], lhsT=ones_s[:], rhs=m[:],
                                 start=True, stop=True)
                pos = cpool.tile([P, NF], f32, tag="pos", name=f"pos{q}")
                nc.vector.tensor_copy(pos[:], psA[:])
                ctot = cpool.tile([P, NF], f32, tag="ctot", name=f"ct{q}")
                nc.vector.tensor_copy(ctot[:], psC[:])
                for f in range(1, NF):
                    nc.vector.tensor_add(
                        ctot[:, f:f + 1], ctot[:, f:f + 1], ctot[:, f - 1:f])
                for f in range(1, NF):
                    nc.vector.tensor_add(
                        pos[:, f:f + 1], pos[:, f:f + 1], ctot[:, f - 1:f])
                # pads: pos -> RT (outside sr range) so no one-hot matches
                nc.vector.tensor_scalar_add(pos[:], pos[:], float(-RT))
                nc.vector.tensor_mul(pos[:], pos[:], m[:])
                nc.vector.tensor_scalar_add(pos[:], pos[:], float(RT))

                # rhs payload per token: [p, f, wgt, mask] (p,f exact in bf16)
                pay = cpool.tile([P, NF, 4], bf16, tag="pay", name=f"pay{q}")
                nc.vector.tensor_copy(pay[:, :, 0], pcol_s[:])
                nc.vector.tensor_copy(pay[:, :, 1], fcol_s[:])
                nc.vector.tensor_copy(pay[:, :, 2], wgt_all[:])
                nc.vector.tensor_copy(pay[:, :, 3], m[:])

                lstf = cpool.tile([P, CT, 4], f32, tag="lst", name=f"lst{q}")
                c0 = 0
                for ct, rows in enumerate(CTS):
                    ps_l = psmall.tile([rows, 4], f32, tag="sm")
                    for f in range(NF):
                        ind = cpool.tile([P, rows], bf16, tag="ind")
                        nc.vector.tensor_tensor(
                            ind[:], pos[:, f:f + 1].to_broadcast([P, rows]),
                            sr_s[:, c0:c0 + rows],
                            mybir.AluOpType.is_equal)
                        nc.tensor.matmul(
                            ps_l[:], lhsT=ind[:], rhs=pay[:, f, :],
                            start=(f == 0), stop=(f == NF - 1))
                    nc.vector.tensor_copy(lstf[0:rows, ct, :], ps_l[:])
                    c0 += rows

                # recover indices; pads (occ=0): gather trash x row, scatter
                # to trash y row.  base = f*128 + p  (pads -> 0)
                base = cpool.tile([P, CT], f32, tag="base", name=f"b{q}")
                nc.vector.tensor_scalar(
                    base[:], lstf[:, :, 1], 128.0, scalar2=None,
                    op0=mybir.AluOpType.mult)
                nc.vector.tensor_add(base[:], base[:], lstf[:, :, 0])
                occ1 = cpool.tile([P, CT], f32, tag="occ1", name=f"o{q}")
                nc.vector.tensor_scalar(
                    occ1[:], lstf[:, :, 3], -1.0, scalar2=None,
                    op0=mybir.AluOpType.add)        # occ-1  (0 or -1)
                # gidx = base + q*RT + (1-occ)*(T - q*RT)
                gidxf = cpool.tile([P, CT], f32, tag="gxf", name=f"gxf{q}")
                nc.vector.tensor_scalar(
                    gidxf[:], occ1[:], -float(T - q * RT), scalar2=None,
                    op0=mybir.AluOpType.mult)
                nc.vector.tensor_add(gidxf[:], gidxf[:], base[:])
                nc.vector.tensor_scalar_add(gidxf[:], gidxf[:], float(q * RT))
                gidx_i = cpool.tile([P, CT], i32, tag="gidx", name=f"gi{q}")
                nc.vector.tensor_copy(gidx_i[:], gidxf[:])
                # yidx = base + (1-occ)*RT
                yidxf = cpool.tile([P, CT], f32, tag="yxf", name=f"yxf{q}")
                nc.vector.tensor_scalar(
                    yidxf[:], occ1[:], -float(RT), scalar2=None,
                    op0=mybir.AluOpType.mult)
                nc.vector.tensor_add(yidxf[:], yidxf[:], base[:])
                yidx_i = cpool.tile([P, CT], i32, tag="yidxi", name=f"yi{q}")
                nc.vector.tensor_copy(yidx_i[:], yidxf[:])
                return lstf, gidx_i, yidx_i

            def gather(q, gidx_i):
                xes = []
                c0 = 0
                for ct, rows in enumerate(CTS):
                    xe = xepool.tile([P, D], f32r, tag="xe")
                    nc.gpsimd.indirect_dma_start(
                        out=xe[0:rows, :],
                        out_offset=None,
                        in_=x_d[:, :],
                        in_offset=bass.IndirectOffsetOnAxis(
                            ap=gidx_i[0:rows, ct:ct + 1], axis=0))
                    xes.append(xe)
                    c0 += rows
                return xes

            def zerofill(q):
                for r in range(YC_ROWS // P):
                    nc.gpsimd.dma_start(ycontribs[q][r * P:(r + 1) * P, :], zt[:])

            # ---------------- prologue ----------------
            # constants (sync queue, small; before gate(0) which uses ident_s)
            ident_s = wpool.tile([P, P], f32r, tag="ident")
            nc.sync.dma_start(ident_s[:], ident_d[:, :])
            utri_s = wpool.tile([P, P], f32, tag="utri")
            nc.sync.dma_start(utri_s[:], utri_d[:, :])
            ones_s = wpool.tile([P, P], f32, tag="ones")
            nc.sync.dma_start(ones_s[:], ones_d[:, :])
            pcol_s = wpool.tile([P, NF], bf16, tag="pcol")
            nc.sync.dma_start(pcol_s[:], pcol_d[:, :])
            fcol_s = wpool.tile([P, NF], bf16, tag="fcol")
            nc.sync.dma_start(fcol_s[:], fcol_d[:, :])
            sr_s = wpool.tile([P, CAP], f32, tag="sr")
            nc.sync.dma_start(sr_s[:], sr_d[:, :])
            zt = wpool.tile([P, D], bf16, tag="zt")
            nc.vector.memset(zt[:], 0.0)

            wgt_next = gate(0)

            # weights: one full-row DMA each (2 KiB descriptor lines), kept
            # off the scalar queue so activations never queue behind them
            w1T_s = wpool.tile([P, DK, I], bf16, tag="w1")
            w3T_s = wpool.tile([P, DK, I], bf16, tag="w3")
            w2T_s = wpool.tile([P, IK, D], bf16, tag="w2")
            nc.sync.dma_start(
                w1T_s[:], w1T_d[:, :].rearrange("(o p) i -> p o i", p=P))
            nc.gpsimd.dma_start(
                w3T_s[:], w3T_d[:, :].rearrange("(o p) i -> p o i", p=P))
            nc.sync.dma_start(
                w2T_s[:], w2T_d[:, :].rearrange("(o p) d -> p o d", p=P))

            zerofill(0)
            zerofill(1)

            lstf, gidx_i, yidx_i = compact(0, wgt_next)
            xes = gather(0, gidx_i)

            # ---------------- the per-range pipeline ----------------
            for q in range(NQ):
                # PE-transpose gathered rows (+cast to bf16)
                xeT = xtpool.tile([P, DK, CAP], bf16, tag="xeT")
                c0 = 0
                for ct, rows in enumerate(CTS):
                    xe = xes[ct]
                    for dk in range(DK):
                        ptr = psmall.tile([P, rows], f32r, tag="sm")
                        nc.tensor.transpose(
                            ptr[:], xe[0:rows, dk * P:(dk + 1) * P],
                            ident_s[0:rows, 0:rows])
                        nc.vector.tensor_copy(
                            xeT[:, dk, c0:c0 + rows], ptr[:])
                    c0 += rows

                if q + 1 < NQ:
                    wgt_next = gate(q + 1)

                # SwiGLU in bf16
                aT = apool.tile([P, IK, CAP], bf16, tag="aT")
                for ik in range(IK):
                    isl = slice(ik * P, (ik + 1) * P)
                    ph = psum.tile([P, CAP], f32, tag="ph")
                    for dk in range(DK):
                        nc.tensor.matmul(
                            ph[:], lhsT=w1T_s[:, dk, isl], rhs=xeT[:, dk, :],
                            start=(dk == 0), stop=(dk == DK - 1))
                    pg = psum.tile([P, CAP], f32, tag="pg")
                    for dk in range(DK):
                        nc.tensor.matmul(
                            pg[:], lhsT=w3T_s[:, dk, isl], rhs=xeT[:, dk, :],
                            start=(dk == 0), stop=(dk == DK - 1))
                    sil = spool.tile([P, CAP], f32r, tag="sil")
                    nc.scalar.activation(
                        sil[:], ph[:], mybir.ActivationFunctionType.Silu)
                    nc.vector.tensor_mul(aT[:, ik, :], sil[:], pg[:])

                # w2 + routing-weight scale + scatter (bf16 rows)
                c0 = 0
                for ct, rows in enumerate(CTS):
                    yt = ypool.tile([P, D], bf16, tag="yt")
                    for dc in range(2):
                        py = pyps.tile([rows, TCH], f32, tag="py")
                        for ik in range(IK):
                            nc.tensor.matmul(
                                py[:],
                                lhsT=aT[:, ik, c0:c0 + rows],
                                rhs=w2T_s[:, ik, dc * TCH:(dc + 1) * TCH],
                                start=(ik == 0), stop=(ik == IK - 1))
                        nc.vector.tensor_scalar_mul(
                            yt[0:rows, dc * TCH:(dc + 1) * TCH], py[:],
                            lstf[0:rows, ct, 2:3])
                    nc.gpsimd.indirect_dma_start(
                        out=ycontribs[q][:, :],
                        out_offset=bass.IndirectOffsetOnAxis(
                            ap=yidx_i[0:rows, ct:ct + 1], axis=0),
                        in_=yt[0:rows, :],
                        in_offset=None)
                    c0 += rows

                # next range's routing + gathers BEFORE this range's RS:
                # the collective occupies the gpsimd queue until it completes
                if q + 1 < NQ:
                    lstf, gidx_i, yidx_i = compact(q + 1, wgt_next)
                    xes = gather(q + 1, gidx_i)
                if q + 2 < NQ:
                    zerofill(q + 2)

                nc.gpsimd.collective_compute(
                    "ReduceScatter",
                    mybir.AluOpType.add,
                    replica_groups=[list(range(NCORES))],
                    ins=[ycontribs[q][0:RT, :].opt()],
                    outs=[yshards[q].opt()],
                )

            # ------------- ship shards to the fp32 output -------------
            for q in range(NQ):
                ysb = ypool.tile([P, D], bf16, tag="ysb")
                nc.sync.dma_start(ysb[:], yshards[q][:])
                yf = ypool.tile([P, D], f32, tag="yf")
                nc.vector.tensor_copy(yf[:], ysb[:])
                nc.sync.dma_start(y_d[q * RSH:(q + 1) * RSH, :], yf[:])
    nc.compile()
    return nc


def _get_nc():
    global _CACHED_NC
    if _CACHED_NC is None:
        _CACHED_NC = _build()
    return _CACHED_NC


def _in_maps(x, gate_w, w1, w3, w2):
    import ml_dtypes
    bf = ml_dtypes.bfloat16
    x = np.asarray(x, dtype=np.float32)
    gate_w = np.asarray(gate_w, dtype=np.float32)
    xT = np.ascontiguousarray(x.T)
    xpad = np.zeros((XPAD_ROWS, D), dtype=np.float32)
    xpad[:T] = x

    # host-side capacity check against the actual gate (cheap, exact)
    s = x @ gate_w.T
    thr = np.sort(s, axis=1)[:, -TOPK]          # 2nd-largest score
    routed = s >= thr[:, None]                  # [T, E]
    cnt = routed.reshape(NQ, RT, E).sum(axis=1)  # [NQ, E]
    if cnt.max() > CAP - 8:
        raise RuntimeError(f"routing capacity exceeded: {cnt.max()} > {CAP}-8")

    utri = np.triu(np.ones((P, P), np.float32), k=1)
    ones = np.ones((P, P), np.float32)
    ident = np.eye(P, dtype=np.float32)
    pcol = np.broadcast_to(np.arange(P, dtype=np.float32)[:, None],
                           (P, NF)).astype(bf)
    fcol = np.broadcast_to(np.arange(NF, dtype=np.float32)[None, :],
                           (P, NF)).astype(bf)
    sr = np.broadcast_to(np.arange(CAP, dtype=np.float32)[None, :],
                         (P, CAP)).copy()

    maps = []
    for e in range(NCORES):
        perm = [e] + [j for j in range(E) if j != e]
        gwT = np.ascontiguousarray(gate_w[perm].T)
        maps.append({
            "xT": xT,
            "x": xpad,
            "gwT": gwT,
            "w1T": np.ascontiguousarray(np.asarray(w1[e], np.float32).T).astype(bf),
            "w3T": np.ascontiguousarray(np.asarray(w3[e], np.float32).T).astype(bf),
            "w2T": np.ascontiguousarray(np.asarray(w2[e], np.float32).T).astype(bf),
            "utri": utri,
            "ones": ones,
            "ident": ident,
            "pcol": pcol,
            "fcol": fcol,
            "sr": sr,
        })
    return maps


def run(x, gate_w, w1, w3, w2, trace=False, trace_cores=None):
    nc = _get_nc()
    maps = _in_maps(x, gate_w, w1, w3, w2)
    res = run_bass_kernel_spmd(
        nc, maps, core_ids=list(range(NCORES)), trace=trace,
        trace_cores=trace_cores)
    # core r's output block q (128 rows) holds tokens [1024q + 128r, +128)
    y = np.empty((T, D), dtype=np.float32)
    for r in range(NCORES):
        yr = res.results[r]["y"]
        for q in range(NQ):
            t0 = q * RT + r * RSH
            y[t0:t0 + RSH] = yr[q * RSH:(q + 1) * RSH]
    return y, res


def kernel(x, gate_w, w1, w3, w2):
    y, _ = run(x, gate_w, w1, w3, w2, trace=False)
    return y.astype(np.float32)


# revision 11
# speedup vs baseline: 1.3709x; 1.1215x over previous
"""MoE SwiGLU (T=4096, D=I=1024, E=8, top-2) on 8 Trainium2 NeuronCores.

Expert-parallel with on-device routing: core e holds expert e's weights
(bf16) in SBUF.  The gate (scores -> softmax -> top-2) is replicated on
every core: matmuls in f32r (single PE pass), softmax math in fp32 on
raw exp values (logits are O(1), no max-shift needed; top-2 selection
on exp is monotone-equivalent).  Each core COMPACTS the token ids
routed to its expert (matmul prefix-sums + one-hot matmuls with exact
small-integer payload in bf16), gathers just those x rows (indirect
DMA), computes SwiGLU in bf16 at full PE rate, scales by the routing
weight, and scatters bf16 rows into a per-range contribution buffer
that was lazily zero-filled.  Four token-range bf16 ReduceScatters
overlap compute; shards are cast back to fp32 on-chip.

Scheduling notes (learned from traces):
- collective_compute and indirect DMA are both gpsimd-queue-only, and a
  collective occupies the queue until it completes.  So range q+1's
  gathers are issued BEFORE RS(q) on that queue.
- the scalar queue carries only activations (Exp/Silu); weight loads
  ride sync/gpsimd as three full-row DMAs (2 KiB descriptor lines).
- the gate for range q+1 is issued between the transposes and SwiGLU of
  range q so the vector softmax hides under the PE matmul stream.
"""
import os
import sys

import numpy as np

for _p in ("/opt/trn_rl_repo", "/root/.axon_site/_ro/trn_rl_repo"):
    if os.path.isdir(_p) and _p not in sys.path:
        sys.path.append(_p)

import concourse.bass as bass  # noqa: E402
import concourse.mybir as mybir  # noqa: E402
import concourse.tile as tile  # noqa: E402
from concourse import bacc  # noqa: E402
from concourse.bass_utils import run_bass_kernel_spmd  # noqa: E402

P = 128
T, D, I, E, TOPK = 4096, 1024, 1024, 8, 2
NCORES = 8
TCH = 512            # gate token chunk (matmul free dim)
DK = D // P          # 8
IK = I // P          # 8
NQ = 4               # ReduceScatter ranges
RT = T // NQ         # 1024 tokens per range
NF = RT // P         # 8 token f-tiles per range
RSH = RT // NCORES   # 128-token shard per core per range
CAP = 320            # routed-token capacity per (core, range); actual max 281
CTS = [128, 128, 64]  # c-tile heights (sum = CAP)
CT = len(CTS)
YC_ROWS = RT + P     # contribution rows + trash row region
XPAD_ROWS = T + P    # x padded with zero rows (gather trash target)
f32 = mybir.dt.float32
f32r = mybir.dt.float32r
bf16 = mybir.dt.bfloat16
i32 = mybir.dt.int32

_CACHED_NC = None


def _build():
    nc = bacc.Bacc("TRN2", target_bir_lowering=False, debug=False,
                   num_devices=NCORES)
    xT_d = nc.dram_tensor("xT", [D, T], f32r, kind="ExternalInput")
    x_d = nc.dram_tensor("x", [XPAD_ROWS, D], f32r, kind="ExternalInput")
    # pre-packed on host: gwTr[p, dk*E + e] = gate_w[perm[e], dk*128 + p]
    gwT_d = nc.dram_tensor("gwTr", [P, DK * E], f32r, kind="ExternalInput")
    w1T_d = nc.dram_tensor("w1T", [D, I], bf16, kind="ExternalInput")
    w3T_d = nc.dram_tensor("w3T", [D, I], bf16, kind="ExternalInput")
    w2T_d = nc.dram_tensor("w2T", [I, D], bf16, kind="ExternalInput")
    utri_d = nc.dram_tensor("utri", [P, P], f32, kind="ExternalInput")
    ones_d = nc.dram_tensor("ones", [P, P], f32, kind="ExternalInput")
    ident_d = nc.dram_tensor("ident", [P, P], f32r, kind="ExternalInput")
    pcol_d = nc.dram_tensor("pcol", [P, NF], bf16, kind="ExternalInput")
    fcol_d = nc.dram_tensor("fcol", [P, NF], bf16, kind="ExternalInput")
    sr_d = nc.dram_tensor("sr", [P, CAP], f32, kind="ExternalInput")
    y_d = nc.dram_tensor("y", [NQ * RSH, D], f32, kind="ExternalOutput")

    with tile.TileContext(nc) as tc:
        with tc.tile_pool(name="wpool", bufs=1) as wpool, \
             tc.tile_pool(name="xgpool", bufs=3) as xgpool, \
             tc.tile_pool(name="gpool", bufs=2) as gpool, \
             tc.tile_pool(name="wapool", bufs=2) as wapool, \
             tc.tile_pool(name="cpool", bufs=2) as cpool, \
             tc.tile_pool(name="xepool", bufs=6) as xepool, \
             tc.tile_pool(name="xtpool", bufs=2) as xtpool, \
             tc.tile_pool(name="apool", bufs=2) as apool, \
             tc.tile_pool(name="spool", bufs=2) as spool, \
             tc.tile_pool(name="ypool", bufs=2) as ypool, \
             tc.tile_pool(name="psum", bufs=2, space="PSUM") as psum, \
             tc.tile_pool(name="pyps", bufs=2, space="PSUM") as pyps, \
             tc.tile_pool(name="psmall", bufs=2, space="PSUM") as psmall, \
             tc.tile_pool(name="dram", bufs=1, space="DRAM") as dram:

            # --- gate weights first (gate(0) needs them immediately);
            # host pre-packed so the DMA is contiguous per partition ---
            gwT_s = wpool.tile([P, DK, E], f32r, tag="gw")
            nc.sync.dma_start(gwT_s[:], gwT_d[:, :])

            ycontribs = [dram.tile([YC_ROWS, D], bf16, tag=f"yc{q}", name=f"yc{q}")
                         for q in range(NQ)]
            yshards = [dram.tile([RSH, D], bf16, tag=f"ys{q}", name=f"ys{q}")
                       for q in range(NQ)]

            # --- gate for one range: f32r matmuls, fp32 softmax on raw exp ---
            def gate(q):
                wgt_all = wapool.tile([P, NF], f32, tag="wgtall", name=f"wa{q}")
                for half in range(2):
                    t0 = q * RT + half * TCH
                    xg_s = xgpool.tile([P, DK, TCH], f32r, tag="xg")
                    nc.sync.dma_start(
                        xg_s[:],
                        xT_d[:, t0:t0 + TCH].rearrange("(o p) t -> p o t", p=P))
                    ps_sT = psmall.tile([E, TCH], f32, tag="sm")
                    for dk in range(DK):
                        nc.tensor.matmul(
                            ps_sT[:], lhsT=gwT_s[:, dk, :], rhs=xg_s[:, dk, :],
                            start=(dk == 0), stop=(dk == DK - 1))
                    sT_sb = gpool.tile([E, TCH], f32r, tag="sTsb")
                    nc.vector.tensor_copy(sT_sb[:], ps_sT[:])
                    for tt in range(4):
                        f = half * 4 + tt
                        ps_g = psmall.tile([P, E], f32r, tag="sm")
                        nc.tensor.transpose(
                            ps_g[:], sT_sb[:, tt * P:(tt + 1) * P],
                            ident_s[:E, :E])
                        exps = gpool.tile([P, E], f32, tag="exps")
                        sumexp = gpool.tile([P, 1], f32, tag="sumexp")
                        nc.scalar.activation(
                            exps[:], ps_g[:], mybir.ActivationFunctionType.Exp,
                            accum_out=sumexp[:, 0:1])
                        mx8 = gpool.tile([P, 8], f32, tag="mx8")
                        nc.vector.max(mx8[:], exps[:])
                        ge = gpool.tile([P, 1], f32, tag="ge")
                        nc.vector.tensor_tensor(
                            ge[:], exps[:, 0:1], mx8[:, 1:2],
                            mybir.AluOpType.is_ge)
                        recip = gpool.tile([P, 1], f32, tag="recip")
                        nc.vector.reciprocal(recip[:], sumexp[:])
                        w0 = gpool.tile([P, 1], f32, tag="w0")
                        nc.vector.tensor_mul(w0[:], exps[:, 0:1], recip[:])
                        nc.vector.tensor_mul(
                            wgt_all[:, f:f + 1], w0[:], ge[:])
                return wgt_all

            # --- compaction: prefix sums + one-hot matmuls (bf16 payload) ---
            def compact(q, wgt_all):
                m = cpool.tile([P, NF], f32, tag="m", name=f"m{q}")
                nc.vector.tensor_scalar(
                    m[:], wgt_all[:], 0.0, scalar2=None,
                    op0=mybir.AluOpType.is_gt)
                psA = psmall.tile([P, NF], f32, tag="sm")
                nc.tensor.matmul(psA[:], lhsT=utri_s[:], rhs=m[:],
                                 start=True, stop=True)
                psC = psmall.tile([P, NF], f32, tag="sm")
                nc.tensor.matmul(psC[:], lhsT=ones_s[:], rhs=m[:],
                                 start=True, stop=True)
                pos = cpool.tile([P, NF], f32, tag="pos", name=f"pos{q}")
                nc.vector.tensor_copy(pos[:], psA[:])
                ctot = cpool.tile([P, NF], f32, tag="ctot", name=f"ct{q}")
                nc.vector.tensor_copy(ctot[:], psC[:])
                for f in range(1, NF):
                    nc.vector.tensor_add(
                        ctot[:, f:f + 1], ctot[:, f:f + 1], ctot[:, f - 1:f])
                for f in range(1, NF):
                    nc.vector.tensor_add(
                        pos[:, f:f + 1], pos[:, f:f + 1], ctot[:, f - 1:f])
                # pads: pos -> RT (outside sr range) so no one-hot matches
                nc.vector.tensor_scalar_add(pos[:], pos[:], float(-RT))
                nc.vector.tensor_mul(pos[:], pos[:], m[:])
                nc.vector.tensor_scalar_add(pos[:], pos[:], float(RT))

                # rhs payload per token: [p, f, wgt, mask] (p,f exact in bf16)
                pay = cpool.tile([P, NF, 4], bf16, tag="pay", name=f"pay{q}")
                nc.vector.tensor_copy(pay[:, :, 0], pcol_s[:])
                nc.vector.tensor_copy(pay[:, :, 1], fcol_s[:])
                nc.vector.tensor_copy(pay[:, :, 2], wgt_all[:])
                nc.vector.tensor_copy(pay[:, :, 3], m[:])

                lstf = cpool.tile([P, CT, 4], f32, tag="lst", name=f"lst{q}")
                c0 = 0
                for ct, rows in enumerate(CTS):
                    ps_l = psmall.tile([rows, 4], f32, tag="sm")
                    for f in range(NF):
                        ind = cpool.tile([P, rows], bf16, tag="ind")
                        nc.vector.tensor_tensor(
                            ind[:], pos[:, f:f + 1].to_broadcast([P, rows]),
                            sr_s[:, c0:c0 + rows],
                            mybir.AluOpType.is_equal)
                        nc.tensor.matmul(
                            ps_l[:], lhsT=ind[:], rhs=pay[:, f, :],
                            start=(f == 0), stop=(f == NF - 1))
                    nc.vector.tensor_copy(lstf[0:rows, ct, :], ps_l[:])
                    c0 += rows

                # recover indices; pads (occ=0): gather trash x row, scatter
                # to trash y row.  base = f*128 + p  (pads -> 0)
                base = cpool.tile([P, CT], f32, tag="base", name=f"b{q}")
                nc.vector.tensor_scalar(
                    base[:], lstf[:, :, 1], 128.0, scalar2=None,
                    op0=mybir.AluOpType.mult)
                nc.vector.tensor_add(base[:], base[:], lstf[:, :, 0])
                occ1 = cpool.tile([P, CT], f32, tag="occ1", name=f"o{q}")
                nc.vector.tensor_scalar(
                    occ1[:], lstf[:, :, 3], -1.0, scalar2=None,
                    op0=mybir.AluOpType.add)        # occ-1  (0 or -1)
                # gidx = base + q*RT + (1-occ)*(T - q*RT)
                gidxf = cpool.tile([P, CT], f32, tag="gxf", name=f"gxf{q}")
                nc.vector.tensor_scalar(
                    gidxf[:], occ1[:], -float(T - q * RT), scalar2=None,
                    op0=mybir.AluOpType.mult)
                nc.vector.tensor_add(gidxf[:], gidxf[:], base[:])
                nc.vector.tensor_scalar_add(gidxf[:], gidxf[:], float(q * RT))
                gidx_i = cpool.tile([P, CT], i32, tag="gidx", name=f"gi{q}")
                nc.vector.tensor_copy(gidx_i[:], gidxf[:])
                # yidx = base + (1-occ)*RT
                yidxf = cpool.tile([P, CT], f32, tag="yxf", name=f"yxf{q}")
                nc.vector.tensor_scalar(
                    yidxf[:], occ1[:], -float(RT), scalar2=None,
                    op0=mybir.AluOpType.mult)
                nc.vector.tensor_add(yidxf[:], yidxf[:], base[:])
                yidx_i = cpool.tile([P, CT], i32, tag="yidxi", name=f"yi{q}")
                nc.vector.tensor_copy(yidx_i[:], yidxf[:])
                return lstf, gidx_i, yidx_i

            def gather(q, gidx_i):
                xes = []
                c0 = 0
                for ct, rows in enumerate(CTS):
                    xe = xepool.tile([P, D], f32r, tag="xe")
                    nc.gpsimd.indirect_dma_start(
                        out=xe[0:rows, :],
                        out_offset=None,
                        in_=x_d[:, :],
                        in_offset=bass.IndirectOffsetOnAxis(
                            ap=gidx_i[0:rows, ct:ct + 1], axis=0))
                    xes.append(xe)
                    c0 += rows
                return xes

            def zerofill(q):
                for r in range(YC_ROWS // P):
                    nc.gpsimd.dma_start(ycontribs[q][r * P:(r + 1) * P, :], zt[:])

            # ---------------- prologue ----------------
            # gate(0) inputs lead the sync queue; everything else follows
            ident_s = wpool.tile([P, P], f32r, tag="ident")
            nc.sync.dma_start(ident_s[:], ident_d[:, :])

            wgt_next = gate(0)

            utri_s = wpool.tile([P, P], f32, tag="utri")
            nc.sync.dma_start(utri_s[:], utri_d[:, :])
            ones_s = wpool.tile([P, P], f32, tag="ones")
            nc.sync.dma_start(ones_s[:], ones_d[:, :])
            pcol_s = wpool.tile([P, NF], bf16, tag="pcol")
            nc.sync.dma_start(pcol_s[:], pcol_d[:, :])
            fcol_s = wpool.tile([P, NF], bf16, tag="fcol")
            nc.sync.dma_start(fcol_s[:], fcol_d[:, :])
            sr_s = wpool.tile([P, CAP], f32, tag="sr")
            nc.sync.dma_start(sr_s[:], sr_d[:, :])
            zt = wpool.tile([P, D], bf16, tag="zt")
            nc.vector.memset(zt[:], 0.0)

            # weights: one full-row DMA each (2 KiB descriptor lines), kept
            # off the scalar queue so activations never queue behind them
            w1T_s = wpool.tile([P, DK, I], bf16, tag="w1")
            w3T_s = wpool.tile([P, DK, I], bf16, tag="w3")
            w2T_s = wpool.tile([P, IK, D], bf16, tag="w2")
            nc.sync.dma_start(
                w1T_s[:], w1T_d[:, :].rearrange("(o p) i -> p o i", p=P))
            nc.gpsimd.dma_start(
                w3T_s[:], w3T_d[:, :].rearrange("(o p) i -> p o i", p=P))
            nc.sync.dma_start(
                w2T_s[:], w2T_d[:, :].rearrange("(o p) d -> p o d", p=P))

            zerofill(0)
            zerofill(1)

            lstf, gidx_i, yidx_i = compact(0, wgt_next)
            xes = gather(0, gidx_i)

            # tiny warmup collective: absorbs the CC-stream cold-start
            # (bootstrap barrier + first-trigger latency) off the real RS0.
            # Placed after gather(0) so it never delays the x gathers.
            warm_in = dram.tile([NCORES, 16], bf16, tag="warmi", name="warmi")
            warm_out = dram.tile([1, 16], bf16, tag="warmo", name="warmo")
            nc.gpsimd.dma_start(warm_in[0:1, :], zt[0:1, 0:16])
            nc.gpsimd.collective_compute(
                "ReduceScatter",
                mybir.AluOpType.add,
                replica_groups=[list(range(NCORES))],
                ins=[warm_in[:, :].opt()],
                outs=[warm_out[:, :].opt()],
            )

            # ---------------- the per-range pipeline ----------------
            for q in range(NQ):
                # PE-transpose gathered rows (+cast to bf16)
                xeT = xtpool.tile([P, DK, CAP], bf16, tag="xeT")
                c0 = 0
                for ct, rows in enumerate(CTS):
                    xe = xes[ct]
                    for dk in range(DK):
                        ptr = psmall.tile([P, rows], f32r, tag="sm")
                        nc.tensor.transpose(
                            ptr[:], xe[0:rows, dk * P:(dk + 1) * P],
                            ident_s[0:rows, 0:rows])
                        nc.vector.tensor_copy(
                            xeT[:, dk, c0:c0 + rows], ptr[:])
                    c0 += rows

                if q + 1 < NQ:
                    wgt_next = gate(q + 1)

                # SwiGLU in bf16
                aT = apool.tile([P, IK, CAP], bf16, tag="aT")
                for ik in range(IK):
                    isl = slice(ik * P, (ik + 1) * P)
                    ph = psum.tile([P, CAP], f32, tag="ph")
                    for dk in range(DK):
                        nc.tensor.matmul(
                            ph[:], lhsT=w1T_s[:, dk, isl], rhs=xeT[:, dk, :],
                            start=(dk == 0), stop=(dk == DK - 1))
                    pg = psum.tile([P, CAP], f32, tag="pg")
                    for dk in range(DK):
                        nc.tensor.matmul(
                            pg[:], lhsT=w3T_s[:, dk, isl], rhs=xeT[:, dk, :],
                            start=(dk == 0), stop=(dk == DK - 1))
                    sil = spool.tile([P, CAP], f32r, tag="sil")
                    nc.scalar.activation(
                        sil[:], ph[:], mybir.ActivationFunctionType.Silu)
                    nc.vector.tensor_mul(aT[:, ik, :], sil[:], pg[:])

                # w2 + routing-weight scale + scatter (bf16 rows)
                c0 = 0
                for ct, rows in enumerate(CTS):
                    yt = ypool.tile([P, D], bf16, tag="yt")
                    for dc in range(2):
                        py = pyps.tile([rows, TCH], f32, tag="py")
                        for ik in range(IK):
                            nc.tensor.matmul(
                                py[:],
                                lhsT=aT[:, ik, c0:c0 + rows],
                                rhs=w2T_s[:, ik, dc * TCH:(dc + 1) * TCH],
                                start=(ik == 0), stop=(ik == IK - 1))
                        nc.vector.tensor_scalar_mul(
                            yt[0:rows, dc * TCH:(dc + 1) * TCH], py[:],
                            lstf[0:rows, ct, 2:3])
                    nc.gpsimd.indirect_dma_start(
                        out=ycontribs[q][:, :],
                        out_offset=bass.IndirectOffsetOnAxis(
                            ap=yidx_i[0:rows, ct:ct + 1], axis=0),
                        in_=yt[0:rows, :],
                        in_offset=None)
                    c0 += rows

                # next range's routing + gathers BEFORE this range's RS:
                # the collective occupies the gpsimd queue until it completes
                if q + 1 < NQ:
                    lstf, gidx_i, yidx_i = compact(q + 1, wgt_next)
                    xes = gather(q + 1, gidx_i)
                if q + 2 < NQ:
                    zerofill(q + 2)

                nc.gpsimd.collective_compute(
                    "ReduceScatter",
                    mybir.AluOpType.add,
                    replica_groups=[list(range(NCORES))],
                    ins=[ycontribs[q][0:RT, :].opt()],
                    outs=[yshards[q].opt()],
                )
                # shard -> fp32 output: gpsimd DMA casts DRAM->DRAM, and the
                # gpsimd queue is already serialized behind the RS, so this
                # touches no other engine (a vector-side cast here stalled
                # the whole pipeline in an earlier revision)
                nc.gpsimd.dma_start(y_d[q * RSH:(q + 1) * RSH, :],
                                    yshards[q][:])
    nc.compile()
    return nc


def _get_nc():
    global _CACHED_NC
    if _CACHED_NC is None:
        _CACHED_NC = _build()
    return _CACHED_NC


def _in_maps(x, gate_w, w1, w3, w2):
    import ml_dtypes
    bf = ml_dtypes.bfloat16
    x = np.asarray(x, dtype=np.float32)
    gate_w = np.asarray(gate_w, dtype=np.float32)
    xT = np.ascontiguousarray(x.T)
    xpad = np.zeros((XPAD_ROWS, D), dtype=np.float32)
    xpad[:T] = x

    # host-side capacity check against the actual gate (cheap, exact)
    s = x @ gate_w.T
    thr = np.sort(s, axis=1)[:, -TOPK]          # 2nd-largest score
    routed = s >= thr[:, None]                  # [T, E]
    cnt = routed.reshape(NQ, RT, E).sum(axis=1)  # [NQ, E]
    if cnt.max() > CAP - 8:
        raise RuntimeError(f"routing capacity exceeded: {cnt.max()} > {CAP}-8")

    utri = np.triu(np.ones((P, P), np.float32), k=1)
    ones = np.ones((P, P), np.float32)
    ident = np.eye(P, dtype=np.float32)
    pcol = np.broadcast_to(np.arange(P, dtype=np.float32)[:, None],
                           (P, NF)).astype(bf)
    fcol = np.broadcast_to(np.arange(NF, dtype=np.float32)[None, :],
                           (P, NF)).astype(bf)
    sr = np.broadcast_to(np.arange(CAP, dtype=np.float32)[None, :],
                         (P, CAP)).copy()

    maps = []
    for e in range(NCORES):
        perm = [e] + [j for j in range(E) if j != e]
        # pre-packed [P, DK*E]: gwTr[p, dk*E + e'] = gate_w[perm[e'], dk*128+p]
        gwTr = np.ascontiguousarray(
            gate_w[perm].T.reshape(DK, P, E).transpose(1, 0, 2).reshape(P, DK * E))
        maps.append({
            "xT": xT,
            "x": xpad,
            "gwTr": gwTr,
            "w1T": np.ascontiguousarray(np.asarray(w1[e], np.float32).T).astype(bf),
            "w3T": np.ascontiguousarray(np.asarray(w3[e], np.float32).T).astype(bf),
            "w2T": np.ascontiguousarray(np.asarray(w2[e], np.float32).T).astype(bf),
            "utri": utri,
            "ones": ones,
            "ident": ident,
            "pcol": pcol,
            "fcol": fcol,
            "sr": sr,
        })
    return maps


def run(x, gate_w, w1, w3, w2, trace=False, trace_cores=None):
    nc = _get_nc()
    maps = _in_maps(x, gate_w, w1, w3, w2)
    res = run_bass_kernel_spmd(
        nc, maps, core_ids=list(range(NCORES)), trace=trace,
        trace_cores=trace_cores)
    # core r's output block q (128 rows) holds tokens [1024q + 128r, +128)
    y = np.empty((T, D), dtype=np.float32)
    for r in range(NCORES):
        yr = res.results[r]["y"]
        for q in range(NQ):
            t0 = q * RT + r * RSH
            y[t0:t0 + RSH] = yr[q * RSH:(q + 1) * RSH]
    return y, res


def kernel(x, gate_w, w1, w3, w2):
    y, _ = run(x, gate_w, w1, w3, w2, trace=False)
    return y.astype(np.float32)


# revision 17
# speedup vs baseline: 1.4497x; 1.0575x over previous
"""MoE SwiGLU (T=4096, D=I=1024, E=8, top-2) on 8 Trainium2 NeuronCores.

Expert-parallel with on-device routing: core e holds expert e's weights
(bf16) in SBUF.  The gate (scores -> softmax -> top-2) is replicated on
every core: matmuls in f32r (single PE pass), softmax math in fp32 on
raw exp values (logits are O(1), no max-shift needed; top-2 selection
on exp is monotone-equivalent).  Each core COMPACTS the token ids
routed to its expert (matmul prefix-sums + one-hot matmuls with exact
small-integer payload in bf16), gathers just those x rows (indirect
DMA), computes SwiGLU in bf16 at full PE rate, scales by the routing
weight, and scatters bf16 rows into a per-range contribution buffer
that was lazily zero-filled.  Four token-range bf16 ReduceScatters
overlap compute; shards are cast back to fp32 on-chip.

Scheduling notes (learned from traces):
- collective_compute and indirect DMA are both gpsimd-queue-only, and a
  collective occupies the queue until it completes.  So range q+1's
  gathers are issued BEFORE RS(q) on that queue.
- the scalar queue carries only activations (Exp/Silu); weight loads
  ride sync/gpsimd as three full-row DMAs (2 KiB descriptor lines).
- the gate for range q+1 is issued between the transposes and SwiGLU of
  range q so the vector softmax hides under the PE matmul stream.
"""
import os
import sys

import numpy as np

for _p in ("/opt/trn_rl_repo", "/root/.axon_site/_ro/trn_rl_repo"):
    if os.path.isdir(_p) and _p not in sys.path:
        sys.path.append(_p)

import concourse.bass as bass  # noqa: E402
import concourse.mybir as mybir  # noqa: E402
import concourse.tile as tile  # noqa: E402
from concourse import bacc  # noqa: E402
from concourse.bass_utils import run_bass_kernel_spmd  # noqa: E402

P = 128
T, D, I, E, TOPK = 4096, 1024, 1024, 8, 2
NCORES = 8
TCH = 512            # gate token chunk (matmul free dim)
DK = D // P          # 8
IK = I // P          # 8
NQ = 4               # ReduceScatter ranges
RT = T // NQ         # 1024 tokens per range
NF = RT // P         # 8 token f-tiles per range
RSH = RT // NCORES   # 128-token shard per core per range
CAP = 384            # routed-token capacity per (core, range); actual max 281
                     # (320 with a ragged 64-slot tile measured SLOWER on the
                     # PE: N=320 matmuls ran ~1.4ns/col vs N=384's ~0.9)
CTS = [128, 128, 128]  # c-tile heights (sum = CAP)
CT = len(CTS)
YC_ROWS = RT + P     # contribution rows + trash row region
XPAD_ROWS = T + P    # x padded with zero rows (gather trash target)
f32 = mybir.dt.float32
f32r = mybir.dt.float32r
bf16 = mybir.dt.bfloat16
i32 = mybir.dt.int32

_CACHED_NC = None


def _build():
    nc = bacc.Bacc("TRN2", target_bir_lowering=False, debug=False,
                   num_devices=NCORES)
    xT_d = nc.dram_tensor("xT", [D, T], f32r, kind="ExternalInput")
    x_d = nc.dram_tensor("x", [XPAD_ROWS, D], f32r, kind="ExternalInput")
    # pre-packed on host: gwTr[p, dk*E + e] = gate_w[perm[e], dk*128 + p]
    gwT_d = nc.dram_tensor("gwTr", [P, DK * E], f32r, kind="ExternalInput")
    w1T_d = nc.dram_tensor("w1T", [D, I], bf16, kind="ExternalInput")
    w3T_d = nc.dram_tensor("w3T", [D, I], bf16, kind="ExternalInput")
    w2T_d = nc.dram_tensor("w2T", [I, D], bf16, kind="ExternalInput")
    utri_d = nc.dram_tensor("utri", [P, P], f32, kind="ExternalInput")
    ones_d = nc.dram_tensor("ones", [P, P], f32, kind="ExternalInput")
    ident_d = nc.dram_tensor("ident", [P, P], f32r, kind="ExternalInput")
    pcol_d = nc.dram_tensor("pcol", [P, NF], bf16, kind="ExternalInput")
    fcol_d = nc.dram_tensor("fcol", [P, NF], bf16, kind="ExternalInput")
    sr_d = nc.dram_tensor("sr", [P, CAP], f32, kind="ExternalInput")
    y_d = nc.dram_tensor("y", [NQ * RSH, D], f32, kind="ExternalOutput")

    with tile.TileContext(nc) as tc:
        with tc.tile_pool(name="wpool", bufs=1) as wpool, \
             tc.tile_pool(name="xgpool", bufs=3) as xgpool, \
             tc.tile_pool(name="gpool", bufs=2) as gpool, \
             tc.tile_pool(name="wapool", bufs=2) as wapool, \
             tc.tile_pool(name="cpool", bufs=2) as cpool, \
             tc.tile_pool(name="xepool", bufs=6) as xepool, \
             tc.tile_pool(name="xtpool", bufs=2) as xtpool, \
             tc.tile_pool(name="apool", bufs=2) as apool, \
             tc.tile_pool(name="spool", bufs=2) as spool, \
             tc.tile_pool(name="ypool", bufs=2) as ypool, \
             tc.tile_pool(name="psum", bufs=2, space="PSUM") as psum, \
             tc.tile_pool(name="pyps", bufs=2, space="PSUM") as pyps, \
             tc.tile_pool(name="psmall", bufs=2, space="PSUM") as psmall, \
             tc.tile_pool(name="dram", bufs=1, space="DRAM") as dram:

            # --- gate weights first (gate(0) needs them immediately);
            # host pre-packed so the DMA is contiguous per partition ---
            gwT_s = wpool.tile([P, DK, E], f32r, tag="gw")
            nc.sync.dma_start(gwT_s[:], gwT_d[:, :])

            ycontribs = [dram.tile([YC_ROWS, D], bf16, tag=f"yc{q}", name=f"yc{q}")
                         for q in range(NQ)]
            # (Shared-scratchpad outputs are unsupported for ReduceScatter)
            yshards = [dram.tile([RSH, D], bf16, tag=f"ys{q}", name=f"ys{q}")
                       for q in range(NQ)]

            # --- gate for one range: f32r matmuls, fp32 softmax on raw exp ---
            def gate(q):
                wgt_all = wapool.tile([P, NF], f32, tag="wgtall", name=f"wa{q}")
                for half in range(2):
                    t0 = q * RT + half * TCH
                    xg_s = xgpool.tile([P, DK, TCH], f32r, tag="xg")
                    nc.sync.dma_start(
                        xg_s[:],
                        xT_d[:, t0:t0 + TCH].rearrange("(o p) t -> p o t", p=P))
                    ps_sT = psmall.tile([E, TCH], f32, tag="sm")
                    for dk in range(DK):
                        nc.tensor.matmul(
                            ps_sT[:], lhsT=gwT_s[:, dk, :], rhs=xg_s[:, dk, :],
                            start=(dk == 0), stop=(dk == DK - 1))
                    sT_sb = gpool.tile([E, TCH], f32r, tag="sTsb")
                    nc.vector.tensor_copy(sT_sb[:], ps_sT[:])
                    for tt in range(4):
                        f = half * 4 + tt
                        ps_g = psmall.tile([P, E], f32r, tag="sm")
                        nc.tensor.transpose(
                            ps_g[:], sT_sb[:, tt * P:(tt + 1) * P],
                            ident_s[:E, :E])
                        exps = gpool.tile([P, E], f32, tag="exps")
                        sumexp = gpool.tile([P, 1], f32, tag="sumexp")
                        nc.scalar.activation(
                            exps[:], ps_g[:], mybir.ActivationFunctionType.Exp,
                            accum_out=sumexp[:, 0:1])
                        mx8 = gpool.tile([P, 8], f32, tag="mx8")
                        nc.vector.max(mx8[:], exps[:])
                        ge = gpool.tile([P, 1], f32, tag="ge")
                        nc.vector.tensor_tensor(
                            ge[:], exps[:, 0:1], mx8[:, 1:2],
                            mybir.AluOpType.is_ge)
                        recip = gpool.tile([P, 1], f32, tag="recip")
                        nc.vector.reciprocal(recip[:], sumexp[:])
                        w0 = gpool.tile([P, 1], f32, tag="w0")
                        nc.vector.tensor_mul(w0[:], exps[:, 0:1], recip[:])
                        nc.vector.tensor_mul(
                            wgt_all[:, f:f + 1], w0[:], ge[:])
                return wgt_all

            # --- compaction: prefix sums + one-hot matmuls (bf16 payload) ---
            def compact(q, wgt_all):
                m = cpool.tile([P, NF], f32, tag="m", name=f"m{q}")
                nc.vector.tensor_scalar(
                    m[:], wgt_all[:], 0.0, scalar2=None,
                    op0=mybir.AluOpType.is_gt)
                psA = psmall.tile([P, NF], f32, tag="sm")
                nc.tensor.matmul(psA[:], lhsT=utri_s[:], rhs=m[:],
                                 start=True, stop=True)
                psC = psmall.tile([P, NF], f32, tag="sm")
                nc.tensor.matmul(psC[:], lhsT=ones_s[:], rhs=m[:],
                                 start=True, stop=True)
                pos = cpool.tile([P, NF], f32, tag="pos", name=f"pos{q}")
                nc.vector.tensor_copy(pos[:], psA[:])
                ctot = cpool.tile([P, NF], f32, tag="ctot", name=f"ct{q}")
                nc.vector.tensor_copy(ctot[:], psC[:])
                for f in range(1, NF):
                    nc.vector.tensor_add(
                        ctot[:, f:f + 1], ctot[:, f:f + 1], ctot[:, f - 1:f])
                for f in range(1, NF):
                    nc.vector.tensor_add(
                        pos[:, f:f + 1], pos[:, f:f + 1], ctot[:, f - 1:f])
                # pads: pos -> RT (outside sr range) so no one-hot matches
                nc.vector.tensor_scalar_add(pos[:], pos[:], float(-RT))
                nc.vector.tensor_mul(pos[:], pos[:], m[:])
                nc.vector.tensor_scalar_add(pos[:], pos[:], float(RT))

                # rhs payload per token: [p, f, wgt, mask] (p,f exact in bf16)
                pay = cpool.tile([P, NF, 4], bf16, tag="pay", name=f"pay{q}")
                nc.vector.tensor_copy(pay[:, :, 0], pcol_s[:])
                nc.vector.tensor_copy(pay[:, :, 1], fcol_s[:])
                nc.vector.tensor_copy(pay[:, :, 2], wgt_all[:])
                nc.vector.tensor_copy(pay[:, :, 3], m[:])

                lstf = cpool.tile([P, CT, 4], f32, tag="lst", name=f"lst{q}")
                c0 = 0
                for ct, rows in enumerate(CTS):
                    ps_l = psmall.tile([rows, 4], f32, tag="sm")
                    for f in range(NF):
                        ind = cpool.tile([P, rows], bf16, tag="ind")
                        nc.vector.tensor_tensor(
                            ind[:], pos[:, f:f + 1].to_broadcast([P, rows]),
                            sr_s[:, c0:c0 + rows],
                            mybir.AluOpType.is_equal)
                        nc.tensor.matmul(
                            ps_l[:], lhsT=ind[:], rhs=pay[:, f, :],
                            start=(f == 0), stop=(f == NF - 1))
                    nc.vector.tensor_copy(lstf[0:rows, ct, :], ps_l[:])
                    c0 += rows

                # recover indices; pads (occ=0): gather trash x row, scatter
                # to trash y row.  base = f*128 + p  (pads -> 0)
                base = cpool.tile([P, CT], f32, tag="base", name=f"b{q}")
                nc.vector.tensor_scalar(
                    base[:], lstf[:, :, 1], 128.0, scalar2=None,
                    op0=mybir.AluOpType.mult)
                nc.vector.tensor_add(base[:], base[:], lstf[:, :, 0])
                occ1 = cpool.tile([P, CT], f32, tag="occ1", name=f"o{q}")
                nc.vector.tensor_scalar(
                    occ1[:], lstf[:, :, 3], -1.0, scalar2=None,
                    op0=mybir.AluOpType.add)        # occ-1  (0 or -1)
                # gidx = base + q*RT + (1-occ)*(T - q*RT)
                gidxf = cpool.tile([P, CT], f32, tag="gxf", name=f"gxf{q}")
                nc.vector.tensor_scalar(
                    gidxf[:], occ1[:], -float(T - q * RT), scalar2=None,
                    op0=mybir.AluOpType.mult)
                nc.vector.tensor_add(gidxf[:], gidxf[:], base[:])
                nc.vector.tensor_scalar_add(gidxf[:], gidxf[:], float(q * RT))
                gidx_i = cpool.tile([P, CT], i32, tag="gidx", name=f"gi{q}")
                nc.vector.tensor_copy(gidx_i[:], gidxf[:])
                # yidx = base + (1-occ)*RT
                yidxf = cpool.tile([P, CT], f32, tag="yxf", name=f"yxf{q}")
                nc.vector.tensor_scalar(
                    yidxf[:], occ1[:], -float(RT), scalar2=None,
                    op0=mybir.AluOpType.mult)
                nc.vector.tensor_add(yidxf[:], yidxf[:], base[:])
                yidx_i = cpool.tile([P, CT], i32, tag="yidxi", name=f"yi{q}")
                nc.vector.tensor_copy(yidx_i[:], yidxf[:])
                return lstf, gidx_i, yidx_i

            def gather(q, gidx_i):
                xes = []
                c0 = 0
                for ct, rows in enumerate(CTS):
                    xe = xepool.tile([P, D], f32r, tag="xe")
                    nc.gpsimd.indirect_dma_start(
                        out=xe[0:rows, :],
                        out_offset=None,
                        in_=x_d[:, :],
                        in_offset=bass.IndirectOffsetOnAxis(
                            ap=gidx_i[0:rows, ct:ct + 1], axis=0))
                    xes.append(xe)
                    c0 += rows
                return xes

            def zerofill(q):
                for r in range(YC_ROWS // P):
                    nc.gpsimd.dma_start(ycontribs[q][r * P:(r + 1) * P, :], zt[:])

            # ---------------- prologue ----------------
            # gate(0) inputs lead the sync queue; everything else follows
            ident_s = wpool.tile([P, P], f32r, tag="ident")
            nc.sync.dma_start(ident_s[:], ident_d[:, :])

            wgt_next = gate(0)

            utri_s = wpool.tile([P, P], f32, tag="utri")
            nc.sync.dma_start(utri_s[:], utri_d[:, :])
            ones_s = wpool.tile([P, P], f32, tag="ones")
            nc.sync.dma_start(ones_s[:], ones_d[:, :])
            pcol_s = wpool.tile([P, NF], bf16, tag="pcol")
            nc.sync.dma_start(pcol_s[:], pcol_d[:, :])
            fcol_s = wpool.tile([P, NF], bf16, tag="fcol")
            nc.sync.dma_start(fcol_s[:], fcol_d[:, :])
            sr_s = wpool.tile([P, CAP], f32, tag="sr")
            nc.sync.dma_start(sr_s[:], sr_d[:, :])
            zt = wpool.tile([P, D], bf16, tag="zt")
            nc.vector.memset(zt[:], 0.0)

            # weights: one full-row DMA each (2 KiB descriptor lines), kept
            # off the scalar queue so activations never queue behind them
            w1T_s = wpool.tile([P, DK, I], bf16, tag="w1")
            w3T_s = wpool.tile([P, DK, I], bf16, tag="w3")
            w2T_s = wpool.tile([P, IK, D], bf16, tag="w2")
            nc.sync.dma_start(
                w1T_s[:], w1T_d[:, :].rearrange("(o p) i -> p o i", p=P))
            nc.gpsimd.dma_start(
                w3T_s[:], w3T_d[:, :].rearrange("(o p) i -> p o i", p=P))
            nc.sync.dma_start(
                w2T_s[:], w2T_d[:, :].rearrange("(o p) d -> p o d", p=P))

            zerofill(0)
            zerofill(1)

            lstf, gidx_i, yidx_i = compact(0, wgt_next)
            xes = gather(0, gidx_i)

            # tiny warmup collective: absorbs the CC-stream cold-start
            # (bootstrap barrier + first-trigger latency) off the real RS0.
            # Placed after gather(0) so it never delays the x gathers.
            warm_in = dram.tile([NCORES, 16], bf16, tag="warmi", name="warmi")
            warm_out = dram.tile([1, 16], bf16, tag="warmo", name="warmo")
            nc.gpsimd.dma_start(warm_in[0:1, :], zt[0:1, 0:16])
            nc.gpsimd.collective_compute(
                "ReduceScatter",
                mybir.AluOpType.add,
                replica_groups=[list(range(NCORES))],
                ins=[warm_in[:, :].opt()],
                outs=[warm_out[:, :].opt()],
            )

            # ---------------- the per-range pipeline ----------------
            for q in range(NQ):
                # PE-transpose gathered rows (+cast to bf16)
                xeT = xtpool.tile([P, DK, CAP], bf16, tag="xeT")
                c0 = 0
                for ct, rows in enumerate(CTS):
                    xe = xes[ct]
                    for dk in range(DK):
                        ptr = psmall.tile([P, rows], f32r, tag="sm")
                        nc.tensor.transpose(
                            ptr[:], xe[0:rows, dk * P:(dk + 1) * P],
                            ident_s[0:rows, 0:rows])
                        nc.vector.tensor_copy(
                            xeT[:, dk, c0:c0 + rows], ptr[:])
                    c0 += rows

                if q + 1 < NQ:
                    wgt_next = gate(q + 1)

                # SwiGLU in bf16
                aT = apool.tile([P, IK, CAP], bf16, tag="aT")
                for ik in range(IK):
                    isl = slice(ik * P, (ik + 1) * P)
                    ph = psum.tile([P, CAP], f32, tag="ph")
                    for dk in range(DK):
                        nc.tensor.matmul(
                            ph[:], lhsT=w1T_s[:, dk, isl], rhs=xeT[:, dk, :],
                            start=(dk == 0), stop=(dk == DK - 1))
                    pg = psum.tile([P, CAP], f32, tag="pg")
                    for dk in range(DK):
                        nc.tensor.matmul(
                            pg[:], lhsT=w3T_s[:, dk, isl], rhs=xeT[:, dk, :],
                            start=(dk == 0), stop=(dk == DK - 1))
                    sil = spool.tile([P, CAP], f32r, tag="sil")
                    nc.scalar.activation(
                        sil[:], ph[:], mybir.ActivationFunctionType.Silu)
                    nc.vector.tensor_mul(aT[:, ik, :], sil[:], pg[:])

                # next range's routing + gathers issued BEFORE this range's
                # scatter/RS: the gathers then fire during w2(q) (gidx is
                # ready mid-SwiGLU) and the collective — which occupies the
                # gpsimd queue until it completes — comes last
                if q + 1 < NQ:
                    lstf_n, gidx_n, yidx_n = compact(q + 1, wgt_next)
                    xes = gather(q + 1, gidx_n)

                # w2 + routing-weight scale + scatter (bf16 rows)
                c0 = 0
                for ct, rows in enumerate(CTS):
                    yt = ypool.tile([P, D], bf16, tag="yt")
                    for dc in range(2):
                        py = pyps.tile([rows, TCH], f32, tag="py")
                        for ik in range(IK):
                            nc.tensor.matmul(
                                py[:],
                                lhsT=aT[:, ik, c0:c0 + rows],
                                rhs=w2T_s[:, ik, dc * TCH:(dc + 1) * TCH],
                                start=(ik == 0), stop=(ik == IK - 1))
                        nc.vector.tensor_scalar_mul(
                            yt[0:rows, dc * TCH:(dc + 1) * TCH], py[:],
                            lstf[0:rows, ct, 2:3])
                    nc.gpsimd.indirect_dma_start(
                        out=ycontribs[q][:, :],
                        out_offset=bass.IndirectOffsetOnAxis(
                            ap=yidx_i[0:rows, ct:ct + 1], axis=0),
                        in_=yt[0:rows, :],
                        in_offset=None)
                    c0 += rows

                if q + 1 < NQ:
                    lstf, gidx_i, yidx_i = lstf_n, gidx_n, yidx_n
                if q + 2 < NQ:
                    zerofill(q + 2)

                nc.gpsimd.collective_compute(
                    "ReduceScatter",
                    mybir.AluOpType.add,
                    replica_groups=[list(range(NCORES))],
                    ins=[ycontribs[q][0:RT, :].opt()],
                    outs=[yshards[q][:, :].opt()],
                )
                # shard -> fp32 output: gpsimd DMA casts DRAM->DRAM, and the
                # gpsimd queue is already serialized behind the RS, so this
                # touches no other engine (a vector-side cast here stalled
                # the whole pipeline in an earlier revision)
                nc.gpsimd.dma_start(y_d[q * RSH:(q + 1) * RSH, :],
                                    yshards[q][:, :])
    nc.compile()
    return nc


def _get_nc():
    global _CACHED_NC
    if _CACHED_NC is None:
        _CACHED_NC = _build()
    return _CACHED_NC


def _in_maps(x, gate_w, w1, w3, w2):
    import ml_dtypes
    bf = ml_dtypes.bfloat16
    x = np.asarray(x, dtype=np.float32)
    gate_w = np.asarray(gate_w, dtype=np.float32)
    xT = np.ascontiguousarray(x.T)
    xpad = np.zeros((XPAD_ROWS, D), dtype=np.float32)
    xpad[:T] = x

    # host-side capacity check against the actual gate (cheap, exact)
    s = x @ gate_w.T
    thr = np.sort(s, axis=1)[:, -TOPK]          # 2nd-largest score
    routed = s >= thr[:, None]                  # [T, E]
    cnt = routed.reshape(NQ, RT, E).sum(axis=1)  # [NQ, E]
    if cnt.max() > CAP - 8:
        raise RuntimeError(f"routing capacity exceeded: {cnt.max()} > {CAP}-8")

    utri = np.triu(np.ones((P, P), np.float32), k=1)
    ones = np.ones((P, P), np.float32)
    ident = np.eye(P, dtype=np.float32)
    pcol = np.broadcast_to(np.arange(P, dtype=np.float32)[:, None],
                           (P, NF)).astype(bf)
    fcol = np.broadcast_to(np.arange(NF, dtype=np.float32)[None, :],
                           (P, NF)).astype(bf)
    sr = np.broadcast_to(np.arange(CAP, dtype=np.float32)[None, :],
                         (P, CAP)).copy()

    maps = []
    for e in range(NCORES):
        perm = [e] + [j for j in range(E) if j != e]
        # pre-packed [P, DK*E]: gwTr[p, dk*E + e'] = gate_w[perm[e'], dk*128+p]
        gwTr = np.ascontiguousarray(
            gate_w[perm].T.reshape(DK, P, E).transpose(1, 0, 2).reshape(P, DK * E))
        maps.append({
            "xT": xT,
            "x": xpad,
            "gwTr": gwTr,
            "w1T": np.ascontiguousarray(np.asarray(w1[e], np.float32).T).astype(bf),
            "w3T": np.ascontiguousarray(np.asarray(w3[e], np.float32).T).astype(bf),
            "w2T": np.ascontiguousarray(np.asarray(w2[e], np.float32).T).astype(bf),
            "utri": utri,
            "ones": ones,
            "ident": ident,
            "pcol": pcol,
            "fcol": fcol,
            "sr": sr,
        })
    return maps


def run(x, gate_w, w1, w3, w2, trace=False, trace_cores=None):
    nc = _get_nc()
    maps = _in_maps(x, gate_w, w1, w3, w2)
    res = run_bass_kernel_spmd(
        nc, maps, core_ids=list(range(NCORES)), trace=trace,
        trace_cores=trace_cores)
    # core r's output block q (128 rows) holds tokens [1024q + 128r, +128)
    y = np.empty((T, D), dtype=np.float32)
    for r in range(NCORES):
        yr = res.results[r]["y"]
        for q in range(NQ):
            t0 = q * RT + r * RSH
            y[t0:t0 + RSH] = yr[q * RSH:(q + 1) * RSH]
    return y, res


def kernel(x, gate_w, w1, w3, w2):
    y, _ = run(x, gate_w, w1, w3, w2, trace=False)
    return y.astype(np.float32)


# revision 19
# speedup vs baseline: 1.5518x; 1.0704x over previous
"""MoE SwiGLU (T=4096, D=I=1024, E=8, top-2) on 8 Trainium2 NeuronCores.

Expert-parallel with on-device routing: core e holds expert e's weights
(bf16) in SBUF.  The gate (scores -> softmax -> top-2) is replicated on
every core: matmuls in f32r (single PE pass), softmax math in fp32 on
raw exp values (logits are O(1), no max-shift needed; top-2 selection
on exp is monotone-equivalent).  Each core COMPACTS the token ids
routed to its expert (matmul prefix-sums + one-hot matmuls with exact
small-integer payload in bf16), gathers just those x rows (indirect
DMA), computes SwiGLU in bf16 at full PE rate, scales by the routing
weight, and scatters bf16 rows into a per-range contribution buffer
that was lazily zero-filled.  Four token-range bf16 ReduceScatters
overlap compute; shards are cast back to fp32 on-chip.

Scheduling notes (learned from traces):
- collective_compute and indirect DMA are both gpsimd-queue-only, and a
  collective occupies the queue until it completes.  So range q+1's
  gathers are issued BEFORE RS(q) on that queue.
- the scalar queue carries only activations (Exp/Silu); weight loads
  ride sync/gpsimd as three full-row DMAs (2 KiB descriptor lines).
- the gate for range q+1 is issued between the transposes and SwiGLU of
  range q so the vector softmax hides under the PE matmul stream.
"""
import os
import sys

import numpy as np

for _p in ("/opt/trn_rl_repo", "/root/.axon_site/_ro/trn_rl_repo"):
    if os.path.isdir(_p) and _p not in sys.path:
        sys.path.append(_p)

import concourse.bass as bass  # noqa: E402
import concourse.mybir as mybir  # noqa: E402
import concourse.tile as tile  # noqa: E402
from concourse import bacc  # noqa: E402
from concourse.bass_utils import run_bass_kernel_spmd  # noqa: E402

P = 128
T, D, I, E, TOPK = 4096, 1024, 1024, 8, 2
NCORES = 8
TCH = 512            # gate token chunk (matmul free dim)
DK = D // P          # 8
IK = I // P          # 8
NQ = 4               # ReduceScatter ranges
RT = T // NQ         # 1024 tokens per range
NF = RT // P         # 8 token f-tiles per range
RSH = RT // NCORES   # 128-token shard per core per range
CAP = 384            # routed-token capacity per (core, range); actual max 281
                     # (320 with a ragged 64-slot tile measured SLOWER on the
                     # PE: N=320 matmuls ran ~1.4ns/col vs N=384's ~0.9)
CTS = [128, 128, 128]  # c-tile heights (sum = CAP)
CT = len(CTS)
YC_ROWS = RT + P     # contribution rows + trash row region
XPAD_ROWS = T + P    # x padded with zero rows (gather trash target)
f32 = mybir.dt.float32
f32r = mybir.dt.float32r
bf16 = mybir.dt.bfloat16
i32 = mybir.dt.int32

_CACHED_NC = None


def _build():
    nc = bacc.Bacc("TRN2", target_bir_lowering=False, debug=False,
                   num_devices=NCORES)
    xT_d = nc.dram_tensor("xT", [D, T], f32r, kind="ExternalInput")
    x_d = nc.dram_tensor("x", [XPAD_ROWS, D], f32r, kind="ExternalInput")
    # pre-packed on host: gwTr[p, dk*E + e] = gate_w[perm[e], dk*128 + p]
    gwT_d = nc.dram_tensor("gwTr", [P, DK * E], f32r, kind="ExternalInput")
    w1T_d = nc.dram_tensor("w1T", [D, I], bf16, kind="ExternalInput")
    w3T_d = nc.dram_tensor("w3T", [D, I], bf16, kind="ExternalInput")
    w2T_d = nc.dram_tensor("w2T", [I, D], bf16, kind="ExternalInput")
    utri_d = nc.dram_tensor("utri", [P, P], f32, kind="ExternalInput")
    ones_d = nc.dram_tensor("ones", [P, P], f32, kind="ExternalInput")
    ident_d = nc.dram_tensor("ident", [P, P], f32r, kind="ExternalInput")
    pcol_d = nc.dram_tensor("pcol", [P, NF], bf16, kind="ExternalInput")
    fcol_d = nc.dram_tensor("fcol", [P, NF], bf16, kind="ExternalInput")
    sr_d = nc.dram_tensor("sr", [P, CAP], f32, kind="ExternalInput")
    y_d = nc.dram_tensor("y", [NQ * RSH, D], f32, kind="ExternalOutput")

    with tile.TileContext(nc) as tc:
        with tc.tile_pool(name="wpool", bufs=1) as wpool, \
             tc.tile_pool(name="xgpool", bufs=3) as xgpool, \
             tc.tile_pool(name="gpool", bufs=2) as gpool, \
             tc.tile_pool(name="wapool", bufs=2) as wapool, \
             tc.tile_pool(name="cpool", bufs=2) as cpool, \
             tc.tile_pool(name="xepool", bufs=6) as xepool, \
             tc.tile_pool(name="xtpool", bufs=2) as xtpool, \
             tc.tile_pool(name="apool", bufs=2) as apool, \
             tc.tile_pool(name="spool", bufs=2) as spool, \
             tc.tile_pool(name="ypool", bufs=2) as ypool, \
             tc.tile_pool(name="psum", bufs=2, space="PSUM") as psum, \
             tc.tile_pool(name="pyps", bufs=2, space="PSUM") as pyps, \
             tc.tile_pool(name="psmall", bufs=2, space="PSUM") as psmall, \
             tc.tile_pool(name="dram", bufs=1, space="DRAM") as dram:

            # --- gate weights first (gate(0) needs them immediately);
            # host pre-packed so the DMA is contiguous per partition ---
            gwT_s = wpool.tile([P, DK, E], f32r, tag="gw")
            nc.sync.dma_start(gwT_s[:], gwT_d[:, :])

            ycontribs = [dram.tile([YC_ROWS, D], bf16, tag=f"yc{q}", name=f"yc{q}")
                         for q in range(NQ)]
            # (Shared-scratchpad outputs are unsupported for ReduceScatter)
            yshards = [dram.tile([RSH, D], bf16, tag=f"ys{q}", name=f"ys{q}")
                       for q in range(NQ)]

            # --- gate for one range: f32r matmuls, fp32 softmax on raw exp ---
            def gate(q):
                wgt_all = wapool.tile([P, NF], f32, tag="wgtall", name=f"wa{q}")
                for half in range(2):
                    t0 = q * RT + half * TCH
                    xg_s = xgpool.tile([P, DK, TCH], f32r, tag="xg")
                    nc.sync.dma_start(
                        xg_s[:],
                        xT_d[:, t0:t0 + TCH].rearrange("(o p) t -> p o t", p=P))
                    ps_sT = psmall.tile([E, TCH], f32, tag="sm")
                    for dk in range(DK):
                        nc.tensor.matmul(
                            ps_sT[:], lhsT=gwT_s[:, dk, :], rhs=xg_s[:, dk, :],
                            start=(dk == 0), stop=(dk == DK - 1))
                    sT_sb = gpool.tile([E, TCH], f32r, tag="sTsb")
                    nc.vector.tensor_copy(sT_sb[:], ps_sT[:])
                    for tt in range(4):
                        f = half * 4 + tt
                        ps_g = psmall.tile([P, E], f32r, tag="sm")
                        nc.tensor.transpose(
                            ps_g[:], sT_sb[:, tt * P:(tt + 1) * P],
                            ident_s[:E, :E])
                        exps = gpool.tile([P, E], f32, tag="exps")
                        sumexp = gpool.tile([P, 1], f32, tag="sumexp")
                        nc.scalar.activation(
                            exps[:], ps_g[:], mybir.ActivationFunctionType.Exp,
                            accum_out=sumexp[:, 0:1])
                        mx8 = gpool.tile([P, 8], f32, tag="mx8")
                        nc.vector.max(mx8[:], exps[:])
                        ge = gpool.tile([P, 1], f32, tag="ge")
                        nc.vector.tensor_tensor(
                            ge[:], exps[:, 0:1], mx8[:, 1:2],
                            mybir.AluOpType.is_ge)
                        recip = gpool.tile([P, 1], f32, tag="recip")
                        nc.vector.reciprocal(recip[:], sumexp[:])
                        w0 = gpool.tile([P, 1], f32, tag="w0")
                        nc.vector.tensor_mul(w0[:], exps[:, 0:1], recip[:])
                        nc.vector.tensor_mul(
                            wgt_all[:, f:f + 1], w0[:], ge[:])
                return wgt_all

            # --- compaction: prefix sums + one-hot matmuls (bf16 payload) ---
            def compact(q, wgt_all):
                m = cpool.tile([P, NF], f32, tag="m", name=f"m{q}")
                nc.vector.tensor_scalar(
                    m[:], wgt_all[:], 0.0, scalar2=None,
                    op0=mybir.AluOpType.is_gt)
                psA = psmall.tile([P, NF], f32, tag="sm")
                nc.tensor.matmul(psA[:], lhsT=utri_s[:], rhs=m[:],
                                 start=True, stop=True)
                psC = psmall.tile([P, NF], f32, tag="sm")
                nc.tensor.matmul(psC[:], lhsT=ones_s[:], rhs=m[:],
                                 start=True, stop=True)
                pos = cpool.tile([P, NF], f32, tag="pos", name=f"pos{q}")
                nc.vector.tensor_copy(pos[:], psA[:])
                ctot = cpool.tile([P, NF], f32, tag="ctot", name=f"ct{q}")
                nc.vector.tensor_copy(ctot[:], psC[:])
                for f in range(1, NF):
                    nc.vector.tensor_add(
                        ctot[:, f:f + 1], ctot[:, f:f + 1], ctot[:, f - 1:f])
                for f in range(1, NF):
                    nc.vector.tensor_add(
                        pos[:, f:f + 1], pos[:, f:f + 1], ctot[:, f - 1:f])
                # pads: pos -> RT (outside sr range) so no one-hot matches
                nc.vector.tensor_scalar_add(pos[:], pos[:], float(-RT))
                nc.vector.tensor_mul(pos[:], pos[:], m[:])
                nc.vector.tensor_scalar_add(pos[:], pos[:], float(RT))

                # rhs payload per token: [p, f, wgt, mask] (p,f exact in bf16)
                pay = cpool.tile([P, NF, 4], bf16, tag="pay", name=f"pay{q}")
                nc.vector.tensor_copy(pay[:, :, 0], pcol_s[:])
                nc.vector.tensor_copy(pay[:, :, 1], fcol_s[:])
                nc.vector.tensor_copy(pay[:, :, 2], wgt_all[:])
                nc.vector.tensor_copy(pay[:, :, 3], m[:])

                lstf = cpool.tile([P, CT, 4], f32, tag="lst", name=f"lst{q}")
                c0 = 0
                for ct, rows in enumerate(CTS):
                    ps_l = psmall.tile([rows, 4], f32, tag="sm")
                    for f in range(NF):
                        ind = cpool.tile([P, rows], bf16, tag="ind")
                        nc.vector.tensor_tensor(
                            ind[:], pos[:, f:f + 1].to_broadcast([P, rows]),
                            sr_s[:, c0:c0 + rows],
                            mybir.AluOpType.is_equal)
                        nc.tensor.matmul(
                            ps_l[:], lhsT=ind[:], rhs=pay[:, f, :],
                            start=(f == 0), stop=(f == NF - 1))
                    nc.vector.tensor_copy(lstf[0:rows, ct, :], ps_l[:])
                    c0 += rows

                # recover indices; pads (occ=0): gather trash x row, scatter
                # to trash y row.  base = f*128 + p  (pads -> 0)
                base = cpool.tile([P, CT], f32, tag="base", name=f"b{q}")
                nc.vector.tensor_scalar(
                    base[:], lstf[:, :, 1], 128.0, scalar2=None,
                    op0=mybir.AluOpType.mult)
                nc.vector.tensor_add(base[:], base[:], lstf[:, :, 0])
                occ1 = cpool.tile([P, CT], f32, tag="occ1", name=f"o{q}")
                nc.vector.tensor_scalar(
                    occ1[:], lstf[:, :, 3], -1.0, scalar2=None,
                    op0=mybir.AluOpType.add)        # occ-1  (0 or -1)
                # gidx = base + q*RT + (1-occ)*(T - q*RT)
                gidxf = cpool.tile([P, CT], f32, tag="gxf", name=f"gxf{q}")
                nc.vector.tensor_scalar(
                    gidxf[:], occ1[:], -float(T - q * RT), scalar2=None,
                    op0=mybir.AluOpType.mult)
                nc.vector.tensor_add(gidxf[:], gidxf[:], base[:])
                nc.vector.tensor_scalar_add(gidxf[:], gidxf[:], float(q * RT))
                gidx_i = cpool.tile([P, CT], i32, tag="gidx", name=f"gi{q}")
                nc.vector.tensor_copy(gidx_i[:], gidxf[:])
                # yidx = base + (1-occ)*RT
                yidxf = cpool.tile([P, CT], f32, tag="yxf", name=f"yxf{q}")
                nc.vector.tensor_scalar(
                    yidxf[:], occ1[:], -float(RT), scalar2=None,
                    op0=mybir.AluOpType.mult)
                nc.vector.tensor_add(yidxf[:], yidxf[:], base[:])
                yidx_i = cpool.tile([P, CT], i32, tag="yidxi", name=f"yi{q}")
                nc.vector.tensor_copy(yidx_i[:], yidxf[:])
                return lstf, gidx_i, yidx_i

            def gather(q, gidx_i):
                xes = []
                c0 = 0
                for ct, rows in enumerate(CTS):
                    xe = xepool.tile([P, D], f32r, tag="xe")
                    nc.gpsimd.indirect_dma_start(
                        out=xe[0:rows, :],
                        out_offset=None,
                        in_=x_d[:, :],
                        in_offset=bass.IndirectOffsetOnAxis(
                            ap=gidx_i[0:rows, ct:ct + 1], axis=0))
                    xes.append(xe)
                    c0 += rows
                return xes

            def zerofill(q):
                for r in range(YC_ROWS // P):
                    nc.gpsimd.dma_start(ycontribs[q][r * P:(r + 1) * P, :], zt[:])

            # ---------------- prologue ----------------
            # gate(0) inputs lead the sync queue; everything else follows
            ident_s = wpool.tile([P, P], f32r, tag="ident")
            nc.sync.dma_start(ident_s[:], ident_d[:, :])

            wgt_next = gate(0)

            utri_s = wpool.tile([P, P], f32, tag="utri")
            nc.sync.dma_start(utri_s[:], utri_d[:, :])
            ones_s = wpool.tile([P, P], f32, tag="ones")
            nc.sync.dma_start(ones_s[:], ones_d[:, :])
            pcol_s = wpool.tile([P, NF], bf16, tag="pcol")
            nc.sync.dma_start(pcol_s[:], pcol_d[:, :])
            fcol_s = wpool.tile([P, NF], bf16, tag="fcol")
            nc.sync.dma_start(fcol_s[:], fcol_d[:, :])
            sr_s = wpool.tile([P, CAP], f32, tag="sr")
            nc.sync.dma_start(sr_s[:], sr_d[:, :])
            zt = wpool.tile([P, D], bf16, tag="zt")
            nc.vector.memset(zt[:], 0.0)

            # weights: one full-row DMA each (2 KiB descriptor lines), kept
            # off the scalar queue so activations never queue behind them
            w1T_s = wpool.tile([P, DK, I], bf16, tag="w1")
            w3T_s = wpool.tile([P, DK, I], bf16, tag="w3")
            w2T_s = wpool.tile([P, IK, D], bf16, tag="w2")
            nc.sync.dma_start(
                w1T_s[:], w1T_d[:, :].rearrange("(o p) i -> p o i", p=P))
            nc.gpsimd.dma_start(
                w3T_s[:], w3T_d[:, :].rearrange("(o p) i -> p o i", p=P))
            nc.sync.dma_start(
                w2T_s[:], w2T_d[:, :].rearrange("(o p) d -> p o d", p=P))

            zerofill(0)
            zerofill(1)

            lstf, gidx_i, yidx_i = compact(0, wgt_next)
            xes = gather(0, gidx_i)

            # tiny warmup collective: absorbs the CC-stream cold-start
            # (bootstrap barrier + first-trigger latency) off the real RS0.
            # Placed after gather(0) so it never delays the x gathers.
            warm_in = dram.tile([NCORES, 16], bf16, tag="warmi", name="warmi")
            warm_out = dram.tile([1, 16], bf16, tag="warmo", name="warmo")
            nc.gpsimd.dma_start(warm_in[0:1, :], zt[0:1, 0:16])
            nc.gpsimd.collective_compute(
                "ReduceScatter",
                mybir.AluOpType.add,
                replica_groups=[list(range(NCORES))],
                ins=[warm_in[:, :].opt()],
                outs=[warm_out[:, :].opt()],
            )

            # ---------------- the per-range pipeline ----------------
            for q in range(NQ):
                # PE-transpose gathered rows (+cast to bf16)
                xeT = xtpool.tile([P, DK, CAP], bf16, tag="xeT")
                c0 = 0
                for ct, rows in enumerate(CTS):
                    xe = xes[ct]
                    for dk in range(DK):
                        ptr = psmall.tile([P, rows], f32r, tag="sm")
                        nc.tensor.transpose(
                            ptr[:], xe[0:rows, dk * P:(dk + 1) * P],
                            ident_s[0:rows, 0:rows])
                        nc.vector.tensor_copy(
                            xeT[:, dk, c0:c0 + rows], ptr[:])
                    c0 += rows

                if q + 1 < NQ:
                    wgt_next = gate(q + 1)

                # SwiGLU in bf16
                aT = apool.tile([P, IK, CAP], bf16, tag="aT")
                for ik in range(IK):
                    isl = slice(ik * P, (ik + 1) * P)
                    ph = psum.tile([P, CAP], f32, tag="ph")
                    for dk in range(DK):
                        nc.tensor.matmul(
                            ph[:], lhsT=w1T_s[:, dk, isl], rhs=xeT[:, dk, :],
                            start=(dk == 0), stop=(dk == DK - 1))
                    pg = psum.tile([P, CAP], f32, tag="pg")
                    for dk in range(DK):
                        nc.tensor.matmul(
                            pg[:], lhsT=w3T_s[:, dk, isl], rhs=xeT[:, dk, :],
                            start=(dk == 0), stop=(dk == DK - 1))
                    sil = spool.tile([P, CAP], f32r, tag="sil")
                    nc.scalar.activation(
                        sil[:], ph[:], mybir.ActivationFunctionType.Silu)
                    nc.vector.tensor_mul(aT[:, ik, :], sil[:], pg[:])

                # next range's routing + gathers issued BEFORE this range's
                # scatter/RS: the gathers then fire during w2(q) (gidx is
                # ready mid-SwiGLU) and the collective — which occupies the
                # gpsimd queue until it completes — comes last
                if q + 1 < NQ:
                    lstf_n, gidx_n, yidx_n = compact(q + 1, wgt_next)
                    xes = gather(q + 1, gidx_n)

                # w2 + routing-weight scale + scatter (bf16 rows)
                c0 = 0
                for ct, rows in enumerate(CTS):
                    yt = ypool.tile([P, D], bf16, tag="yt")
                    for dc in range(2):
                        py = pyps.tile([rows, TCH], f32, tag="py")
                        for ik in range(IK):
                            nc.tensor.matmul(
                                py[:],
                                lhsT=aT[:, ik, c0:c0 + rows],
                                rhs=w2T_s[:, ik, dc * TCH:(dc + 1) * TCH],
                                start=(ik == 0), stop=(ik == IK - 1))
                        # scalar engine, NOT vector: the vector queue at this
                        # point is full of compact(q+1) work, and a vector
                        # scale here stalls the py PSUM ring (and the PE)
                        nc.scalar.mul(
                            yt[0:rows, dc * TCH:(dc + 1) * TCH], py[:],
                            lstf[0:rows, ct, 2:3])
                    nc.gpsimd.indirect_dma_start(
                        out=ycontribs[q][:, :],
                        out_offset=bass.IndirectOffsetOnAxis(
                            ap=yidx_i[0:rows, ct:ct + 1], axis=0),
                        in_=yt[0:rows, :],
                        in_offset=None)
                    c0 += rows

                if q + 1 < NQ:
                    lstf, gidx_i, yidx_i = lstf_n, gidx_n, yidx_n
                if q + 2 < NQ:
                    zerofill(q + 2)

                nc.gpsimd.collective_compute(
                    "ReduceScatter",
                    mybir.AluOpType.add,
                    replica_groups=[list(range(NCORES))],
                    ins=[ycontribs[q][0:RT, :].opt()],
                    outs=[yshards[q][:, :].opt()],
                )
                # shard -> fp32 output: gpsimd DMA casts DRAM->DRAM, and the
                # gpsimd queue is already serialized behind the RS, so this
                # touches no other engine (a vector-side cast here stalled
                # the whole pipeline in an earlier revision)
                nc.gpsimd.dma_start(y_d[q * RSH:(q + 1) * RSH, :],
                                    yshards[q][:, :])
    nc.compile()
    return nc


def _get_nc():
    global _CACHED_NC
    if _CACHED_NC is None:
        _CACHED_NC = _build()
    return _CACHED_NC


def _in_maps(x, gate_w, w1, w3, w2):
    import ml_dtypes
    bf = ml_dtypes.bfloat16
    x = np.asarray(x, dtype=np.float32)
    gate_w = np.asarray(gate_w, dtype=np.float32)
    xT = np.ascontiguousarray(x.T)
    xpad = np.zeros((XPAD_ROWS, D), dtype=np.float32)
    xpad[:T] = x

    # host-side capacity check against the actual gate (cheap, exact)
    s = x @ gate_w.T
    thr = np.sort(s, axis=1)[:, -TOPK]          # 2nd-largest score
    routed = s >= thr[:, None]                  # [T, E]
    cnt = routed.reshape(NQ, RT, E).sum(axis=1)  # [NQ, E]
    if cnt.max() > CAP - 8:
        raise RuntimeError(f"routing capacity exceeded: {cnt.max()} > {CAP}-8")

    utri = np.triu(np.ones((P, P), np.float32), k=1)
    ones = np.ones((P, P), np.float32)
    ident = np.eye(P, dtype=np.float32)
    pcol = np.broadcast_to(np.arange(P, dtype=np.float32)[:, None],
                           (P, NF)).astype(bf)
    fcol = np.broadcast_to(np.arange(NF, dtype=np.float32)[None, :],
                           (P, NF)).astype(bf)
    sr = np.broadcast_to(np.arange(CAP, dtype=np.float32)[None, :],
                         (P, CAP)).copy()

    maps = []
    for e in range(NCORES):
        perm = [e] + [j for j in range(E) if j != e]
        # pre-packed [P, DK*E]: gwTr[p, dk*E + e'] = gate_w[perm[e'], dk*128+p]
        gwTr = np.ascontiguousarray(
            gate_w[perm].T.reshape(DK, P, E).transpose(1, 0, 2).reshape(P, DK * E))
        maps.append({
            "xT": xT,
            "x": xpad,
            "gwTr": gwTr,
            "w1T": np.ascontiguousarray(np.asarray(w1[e], np.float32).T).astype(bf),
            "w3T": np.ascontiguousarray(np.asarray(w3[e], np.float32).T).astype(bf),
            "w2T": np.ascontiguousarray(np.asarray(w2[e], np.float32).T).astype(bf),
            "utri": utri,
            "ones": ones,
            "ident": ident,
            "pcol": pcol,
            "fcol": fcol,
            "sr": sr,
        })
    return maps


def run(x, gate_w, w1, w3, w2, trace=False, trace_cores=None):
    nc = _get_nc()
    maps = _in_maps(x, gate_w, w1, w3, w2)
    res = run_bass_kernel_spmd(
        nc, maps, core_ids=list(range(NCORES)), trace=trace,
        trace_cores=trace_cores)
    # core r's output block q (128 rows) holds tokens [1024q + 128r, +128)
    y = np.empty((T, D), dtype=np.float32)
    for r in range(NCORES):
        yr = res.results[r]["y"]
        for q in range(NQ):
            t0 = q * RT + r * RSH
            y[t0:t0 + RSH] = yr[q * RSH:(q + 1) * RSH]
    return y, res


def kernel(x, gate_w, w1, w3, w2):
    y, _ = run(x, gate_w, w1, w3, w2, trace=False)
    return y.astype(np.float32)


# revision 22
# speedup vs baseline: 1.5656x; 1.0089x over previous
"""MoE SwiGLU (T=4096, D=I=1024, E=8, top-2) on 8 Trainium2 NeuronCores.

Expert-parallel with on-device routing: core e holds expert e's weights
(bf16) in SBUF.  The gate (scores -> softmax -> top-2) is replicated on
every core: matmuls in f32r (single PE pass), softmax math in fp32 on
raw exp values (logits are O(1), no max-shift needed; top-2 selection
on exp is monotone-equivalent).  Each core COMPACTS the token ids
routed to its expert (matmul prefix-sums + one-hot matmuls with exact
small-integer payload in bf16), gathers just those x rows (indirect
DMA), computes SwiGLU in bf16 at full PE rate, scales by the routing
weight, and scatters bf16 rows into a per-range contribution buffer
that was lazily zero-filled.  Four token-range bf16 ReduceScatters
overlap compute; shards are cast back to fp32 on-chip.

Scheduling notes (learned from traces):
- collective_compute and indirect DMA are both gpsimd-queue-only, and a
  collective occupies the queue until it completes.  So range q+1's
  gathers are issued BEFORE RS(q) on that queue.
- the scalar queue carries only activations (Exp/Silu); weight loads
  ride sync/gpsimd as three full-row DMAs (2 KiB descriptor lines).
- the gate for range q+1 is issued between the transposes and SwiGLU of
  range q so the vector softmax hides under the PE matmul stream.
"""
import os
import sys

import numpy as np

for _p in ("/opt/trn_rl_repo", "/root/.axon_site/_ro/trn_rl_repo"):
    if os.path.isdir(_p) and _p not in sys.path:
        sys.path.append(_p)

import concourse.bass as bass  # noqa: E402
import concourse.mybir as mybir  # noqa: E402
import concourse.tile as tile  # noqa: E402
from concourse import bacc  # noqa: E402
from concourse.bass_utils import run_bass_kernel_spmd  # noqa: E402

P = 128
T, D, I, E, TOPK = 4096, 1024, 1024, 8, 2
NCORES = 8
TCH = 512            # gate token chunk (matmul free dim)
DK = D // P          # 8
IK = I // P          # 8
NQ = 4               # ReduceScatter ranges
RT = T // NQ         # 1024 tokens per range
NF = RT // P         # 8 token f-tiles per range
RSH = RT // NCORES   # 128-token shard per core per range
CAP = 384            # routed-token capacity per (core, range); actual max 281
                     # (320 with a ragged 64-slot tile measured SLOWER on the
                     # PE: N=320 matmuls ran ~1.4ns/col vs N=384's ~0.9)
CTS = [128, 128, 128]  # c-tile heights (sum = CAP)
CT = len(CTS)
YC_ROWS = RT + P     # contribution rows + trash row region
XPAD_ROWS = T + P    # x padded with zero rows (gather trash target)
f32 = mybir.dt.float32
f32r = mybir.dt.float32r
bf16 = mybir.dt.bfloat16
i32 = mybir.dt.int32

_CACHED_NC = None


def _build():
    nc = bacc.Bacc("TRN2", target_bir_lowering=False, debug=False,
                   num_devices=NCORES)
    xT_d = nc.dram_tensor("xT", [D, T], f32r, kind="ExternalInput")
    x_d = nc.dram_tensor("x", [XPAD_ROWS, D], f32r, kind="ExternalInput")
    # pre-packed on host: gwTr[p, dk*E + e] = gate_w[perm[e], dk*128 + p]
    gwT_d = nc.dram_tensor("gwTr", [P, DK * E], f32r, kind="ExternalInput")
    w1T_d = nc.dram_tensor("w1T", [D, I], bf16, kind="ExternalInput")
    w3T_d = nc.dram_tensor("w3T", [D, I], bf16, kind="ExternalInput")
    w2T_d = nc.dram_tensor("w2T", [I, D], bf16, kind="ExternalInput")
    utri_d = nc.dram_tensor("utri", [P, P], f32, kind="ExternalInput")
    ones_d = nc.dram_tensor("ones", [P, P], f32, kind="ExternalInput")
    ident_d = nc.dram_tensor("ident", [P, P], f32r, kind="ExternalInput")
    pcol_d = nc.dram_tensor("pcol", [P, NF], bf16, kind="ExternalInput")
    fcol_d = nc.dram_tensor("fcol", [P, NF], bf16, kind="ExternalInput")
    sr_d = nc.dram_tensor("sr", [P, CAP], f32, kind="ExternalInput")
    y_d = nc.dram_tensor("y", [NQ * RSH, D], f32, kind="ExternalOutput")

    with tile.TileContext(nc) as tc:
        with tc.tile_pool(name="wpool", bufs=1) as wpool, \
             tc.tile_pool(name="xgpool", bufs=3) as xgpool, \
             tc.tile_pool(name="gpool", bufs=2) as gpool, \
             tc.tile_pool(name="wapool", bufs=2) as wapool, \
             tc.tile_pool(name="cpool", bufs=2) as cpool, \
             tc.tile_pool(name="xepool", bufs=6) as xepool, \
             tc.tile_pool(name="xtpool", bufs=2) as xtpool, \
             tc.tile_pool(name="apool", bufs=2) as apool, \
             tc.tile_pool(name="spool", bufs=2) as spool, \
             tc.tile_pool(name="ypool", bufs=2) as ypool, \
             tc.tile_pool(name="psum", bufs=2, space="PSUM") as psum, \
             tc.tile_pool(name="pyps", bufs=2, space="PSUM") as pyps, \
             tc.tile_pool(name="psmall", bufs=2, space="PSUM") as psmall, \
             tc.tile_pool(name="dram", bufs=1, space="DRAM") as dram:

            # --- gate weights first (gate(0) needs them immediately);
            # host pre-packed so the DMA is contiguous per partition ---
            gwT_s = wpool.tile([P, DK, E], f32r, tag="gw")
            nc.sync.dma_start(gwT_s[:], gwT_d[:, :])

            ycontribs = [dram.tile([YC_ROWS, D], bf16, tag=f"yc{q}", name=f"yc{q}")
                         for q in range(NQ)]
            # (Shared-scratchpad outputs are unsupported for ReduceScatter)
            yshards = [dram.tile([RSH, D], bf16, tag=f"ys{q}", name=f"ys{q}")
                       for q in range(NQ)]

            # --- gate for one range: f32r matmuls, fp32 softmax on raw exp ---
            def gate(q):
                wgt_all = wapool.tile([P, NF], f32, tag="wgtall", name=f"wa{q}")
                for half in range(2):
                    t0 = q * RT + half * TCH
                    xg_s = xgpool.tile([P, DK, TCH], f32r, tag="xg")
                    nc.sync.dma_start(
                        xg_s[:],
                        xT_d[:, t0:t0 + TCH].rearrange("(o p) t -> p o t", p=P))
                    # score PSUM rides the (idle-here) ph ring, not the
                    # crowded "sm" ring, so the two chunks' matmuls pipeline
                    ps_sT = psum.tile([E, TCH], f32, tag="ph")
                    for dk in range(DK):
                        nc.tensor.matmul(
                            ps_sT[:], lhsT=gwT_s[:, dk, :], rhs=xg_s[:, dk, :],
                            start=(dk == 0), stop=(dk == DK - 1))
                    sT_sb = gpool.tile([E, TCH], f32r, tag="sTsb")
                    nc.vector.tensor_copy(sT_sb[:], ps_sT[:])
                    for tt in range(4):
                        f = half * 4 + tt
                        ps_g = psmall.tile([P, E], f32r, tag="sm")
                        nc.tensor.transpose(
                            ps_g[:], sT_sb[:, tt * P:(tt + 1) * P],
                            ident_s[:E, :E])
                        exps = gpool.tile([P, E], f32, tag="exps")
                        sumexp = gpool.tile([P, 1], f32, tag="sumexp")
                        nc.scalar.activation(
                            exps[:], ps_g[:], mybir.ActivationFunctionType.Exp,
                            accum_out=sumexp[:, 0:1])
                        mx8 = gpool.tile([P, 8], f32, tag="mx8")
                        nc.vector.max(mx8[:], exps[:])
                        ge = gpool.tile([P, 1], f32, tag="ge")
                        nc.vector.tensor_tensor(
                            ge[:], exps[:, 0:1], mx8[:, 1:2],
                            mybir.AluOpType.is_ge)
                        recip = gpool.tile([P, 1], f32, tag="recip")
                        nc.vector.reciprocal(recip[:], sumexp[:])
                        w0 = gpool.tile([P, 1], f32, tag="w0")
                        nc.vector.tensor_mul(w0[:], exps[:, 0:1], recip[:])
                        nc.vector.tensor_mul(
                            wgt_all[:, f:f + 1], w0[:], ge[:])
                return wgt_all

            # --- compaction: prefix sums + one-hot matmuls (bf16 payload) ---
            def compact(q, wgt_all):
                m = cpool.tile([P, NF], f32, tag="m", name=f"m{q}")
                nc.vector.tensor_scalar(
                    m[:], wgt_all[:], 0.0, scalar2=None,
                    op0=mybir.AluOpType.is_gt)
                psA = psmall.tile([P, NF], f32, tag="sm")
                nc.tensor.matmul(psA[:], lhsT=utri_s[:], rhs=m[:],
                                 start=True, stop=True)
                psC = psmall.tile([P, NF], f32, tag="sm")
                nc.tensor.matmul(psC[:], lhsT=ones_s[:], rhs=m[:],
                                 start=True, stop=True)
                pos = cpool.tile([P, NF], f32, tag="pos", name=f"pos{q}")
                nc.vector.tensor_copy(pos[:], psA[:])
                ctot = cpool.tile([P, NF], f32, tag="ctot", name=f"ct{q}")
                nc.vector.tensor_copy(ctot[:], psC[:])
                for f in range(1, NF):
                    nc.vector.tensor_add(
                        ctot[:, f:f + 1], ctot[:, f:f + 1], ctot[:, f - 1:f])
                for f in range(1, NF):
                    nc.vector.tensor_add(
                        pos[:, f:f + 1], pos[:, f:f + 1], ctot[:, f - 1:f])
                # pads: pos -> RT (outside sr range) so no one-hot matches
                nc.vector.tensor_scalar_add(pos[:], pos[:], float(-RT))
                nc.vector.tensor_mul(pos[:], pos[:], m[:])
                nc.vector.tensor_scalar_add(pos[:], pos[:], float(RT))

                # rhs payload per token: [p, f, wgt, mask] (p,f exact in bf16)
                pay = cpool.tile([P, NF, 4], bf16, tag="pay", name=f"pay{q}")
                nc.vector.tensor_copy(pay[:, :, 0], pcol_s[:])
                nc.vector.tensor_copy(pay[:, :, 1], fcol_s[:])
                nc.vector.tensor_copy(pay[:, :, 2], wgt_all[:])
                nc.vector.tensor_copy(pay[:, :, 3], m[:])

                lstf = cpool.tile([P, CT, 4], f32, tag="lst", name=f"lst{q}")
                c0 = 0
                for ct, rows in enumerate(CTS):
                    ps_l = psmall.tile([rows, 4], f32, tag="sm")
                    for f in range(NF):
                        ind = cpool.tile([P, rows], bf16, tag="ind")
                        nc.vector.tensor_tensor(
                            ind[:], pos[:, f:f + 1].to_broadcast([P, rows]),
                            sr_s[:, c0:c0 + rows],
                            mybir.AluOpType.is_equal)
                        nc.tensor.matmul(
                            ps_l[:], lhsT=ind[:], rhs=pay[:, f, :],
                            start=(f == 0), stop=(f == NF - 1))
                    nc.vector.tensor_copy(lstf[0:rows, ct, :], ps_l[:])
                    c0 += rows

                # recover indices; pads (occ=0): gather trash x row, scatter
                # to trash y row.  base = f*128 + p  (pads -> 0)
                base = cpool.tile([P, CT], f32, tag="base", name=f"b{q}")
                nc.vector.tensor_scalar(
                    base[:], lstf[:, :, 1], 128.0, scalar2=None,
                    op0=mybir.AluOpType.mult)
                nc.vector.tensor_add(base[:], base[:], lstf[:, :, 0])
                occ1 = cpool.tile([P, CT], f32, tag="occ1", name=f"o{q}")
                nc.vector.tensor_scalar(
                    occ1[:], lstf[:, :, 3], -1.0, scalar2=None,
                    op0=mybir.AluOpType.add)        # occ-1  (0 or -1)
                # gidx = base + q*RT + (1-occ)*(T - q*RT)
                gidxf = cpool.tile([P, CT], f32, tag="gxf", name=f"gxf{q}")
                nc.vector.tensor_scalar(
                    gidxf[:], occ1[:], -float(T - q * RT), scalar2=None,
                    op0=mybir.AluOpType.mult)
                nc.vector.tensor_add(gidxf[:], gidxf[:], base[:])
                nc.vector.tensor_scalar_add(gidxf[:], gidxf[:], float(q * RT))
                gidx_i = cpool.tile([P, CT], i32, tag="gidx", name=f"gi{q}")
                nc.vector.tensor_copy(gidx_i[:], gidxf[:])
                # yidx = base + (1-occ)*RT
                yidxf = cpool.tile([P, CT], f32, tag="yxf", name=f"yxf{q}")
                nc.vector.tensor_scalar(
                    yidxf[:], occ1[:], -float(RT), scalar2=None,
                    op0=mybir.AluOpType.mult)
                nc.vector.tensor_add(yidxf[:], yidxf[:], base[:])
                yidx_i = cpool.tile([P, CT], i32, tag="yidxi", name=f"yi{q}")
                nc.vector.tensor_copy(yidx_i[:], yidxf[:])
                return lstf, gidx_i, yidx_i

            def gather(q, gidx_i):
                xes = []
                c0 = 0
                for ct, rows in enumerate(CTS):
                    xe = xepool.tile([P, D], f32r, tag="xe")
                    nc.gpsimd.indirect_dma_start(
                        out=xe[0:rows, :],
                        out_offset=None,
                        in_=x_d[:, :],
                        in_offset=bass.IndirectOffsetOnAxis(
                            ap=gidx_i[0:rows, ct:ct + 1], axis=0))
                    xes.append(xe)
                    c0 += rows
                return xes

            def zerofill(q):
                for r in range(YC_ROWS // P):
                    nc.gpsimd.dma_start(ycontribs[q][r * P:(r + 1) * P, :], zt[:])

            # ---------------- prologue ----------------
            # gate(0) inputs lead the sync queue; everything else follows
            ident_s = wpool.tile([P, P], f32r, tag="ident")
            nc.sync.dma_start(ident_s[:], ident_d[:, :])

            wgt_next = gate(0)

            utri_s = wpool.tile([P, P], f32, tag="utri")
            nc.sync.dma_start(utri_s[:], utri_d[:, :])
            ones_s = wpool.tile([P, P], f32, tag="ones")
            nc.sync.dma_start(ones_s[:], ones_d[:, :])
            pcol_s = wpool.tile([P, NF], bf16, tag="pcol")
            nc.sync.dma_start(pcol_s[:], pcol_d[:, :])
            fcol_s = wpool.tile([P, NF], bf16, tag="fcol")
            nc.sync.dma_start(fcol_s[:], fcol_d[:, :])
            sr_s = wpool.tile([P, CAP], f32, tag="sr")
            nc.sync.dma_start(sr_s[:], sr_d[:, :])
            zt = wpool.tile([P, D], bf16, tag="zt")
            nc.vector.memset(zt[:], 0.0)

            # weights: one full-row DMA each (2 KiB descriptor lines), kept
            # off the scalar queue so activations never queue behind them
            w1T_s = wpool.tile([P, DK, I], bf16, tag="w1")
            w3T_s = wpool.tile([P, DK, I], bf16, tag="w3")
            w2T_s = wpool.tile([P, IK, D], bf16, tag="w2")
            nc.sync.dma_start(
                w1T_s[:], w1T_d[:, :].rearrange("(o p) i -> p o i", p=P))
            nc.gpsimd.dma_start(
                w3T_s[:], w3T_d[:, :].rearrange("(o p) i -> p o i", p=P))
            nc.sync.dma_start(
                w2T_s[:], w2T_d[:, :].rearrange("(o p) d -> p o d", p=P))

            zerofill(0)
            zerofill(1)

            lstf, gidx_i, yidx_i = compact(0, wgt_next)
            xes = gather(0, gidx_i)

            # tiny warmup collective: absorbs the CC-stream cold-start
            # (bootstrap barrier + first-trigger latency) off the real RS0.
            # Placed after gather(0) so it never delays the x gathers.
            warm_in = dram.tile([NCORES, 16], bf16, tag="warmi", name="warmi")
            warm_out = dram.tile([1, 16], bf16, tag="warmo", name="warmo")
            nc.gpsimd.dma_start(warm_in[0:1, :], zt[0:1, 0:16])
            nc.gpsimd.collective_compute(
                "ReduceScatter",
                mybir.AluOpType.add,
                replica_groups=[list(range(NCORES))],
                ins=[warm_in[:, :].opt()],
                outs=[warm_out[:, :].opt()],
            )

            # ---------------- the per-range pipeline ----------------
            for q in range(NQ):
                # PE-transpose gathered rows (+cast to bf16)
                xeT = xtpool.tile([P, DK, CAP], bf16, tag="xeT")
                c0 = 0
                for ct, rows in enumerate(CTS):
                    xe = xes[ct]
                    for dk in range(DK):
                        ptr = psmall.tile([P, rows], f32r, tag="sm")
                        nc.tensor.transpose(
                            ptr[:], xe[0:rows, dk * P:(dk + 1) * P],
                            ident_s[0:rows, 0:rows])
                        nc.vector.tensor_copy(
                            xeT[:, dk, c0:c0 + rows], ptr[:])
                    c0 += rows

                if q + 1 < NQ:
                    wgt_next = gate(q + 1)

                # SwiGLU in bf16
                aT = apool.tile([P, IK, CAP], bf16, tag="aT")
                for ik in range(IK):
                    isl = slice(ik * P, (ik + 1) * P)
                    ph = psum.tile([P, CAP], f32, tag="ph")
                    for dk in range(DK):
                        nc.tensor.matmul(
                            ph[:], lhsT=w1T_s[:, dk, isl], rhs=xeT[:, dk, :],
                            start=(dk == 0), stop=(dk == DK - 1))
                    pg = psum.tile([P, CAP], f32, tag="pg")
                    for dk in range(DK):
                        nc.tensor.matmul(
                            pg[:], lhsT=w3T_s[:, dk, isl], rhs=xeT[:, dk, :],
                            start=(dk == 0), stop=(dk == DK - 1))
                    sil = spool.tile([P, CAP], f32r, tag="sil")
                    nc.scalar.activation(
                        sil[:], ph[:], mybir.ActivationFunctionType.Silu)
                    nc.vector.tensor_mul(aT[:, ik, :], sil[:], pg[:])

                # next range's routing + gathers issued BEFORE this range's
                # scatter/RS: the gathers then fire during w2(q) (gidx is
                # ready mid-SwiGLU) and the collective — which occupies the
                # gpsimd queue until it completes — comes last
                if q + 1 < NQ:
                    lstf_n, gidx_n, yidx_n = compact(q + 1, wgt_next)
                    xes = gather(q + 1, gidx_n)

                # w2 + routing-weight scale + scatter (bf16 rows)
                c0 = 0
                for ct, rows in enumerate(CTS):
                    yt = ypool.tile([P, D], bf16, tag="yt")
                    for dc in range(2):
                        py = pyps.tile([rows, TCH], f32, tag="py")
                        for ik in range(IK):
                            nc.tensor.matmul(
                                py[:],
                                lhsT=aT[:, ik, c0:c0 + rows],
                                rhs=w2T_s[:, ik, dc * TCH:(dc + 1) * TCH],
                                start=(ik == 0), stop=(ik == IK - 1))
                        # scalar engine, NOT vector: the vector queue at this
                        # point is full of compact(q+1) work, and a vector
                        # scale here stalls the py PSUM ring (and the PE).
                        # (gpsimd reading PSUM fails to compile)
                        nc.scalar.mul(
                            yt[0:rows, dc * TCH:(dc + 1) * TCH], py[:],
                            lstf[0:rows, ct, 2:3])
                    nc.gpsimd.indirect_dma_start(
                        out=ycontribs[q][:, :],
                        out_offset=bass.IndirectOffsetOnAxis(
                            ap=yidx_i[0:rows, ct:ct + 1], axis=0),
                        in_=yt[0:rows, :],
                        in_offset=None)
                    c0 += rows

                if q + 1 < NQ:
                    lstf, gidx_i, yidx_i = lstf_n, gidx_n, yidx_n
                if q + 2 < NQ:
                    zerofill(q + 2)

                nc.gpsimd.collective_compute(
                    "ReduceScatter",
                    mybir.AluOpType.add,
                    replica_groups=[list(range(NCORES))],
                    ins=[ycontribs[q][0:RT, :].opt()],
                    outs=[yshards[q][:, :].opt()],
                )
                # shard -> fp32 output: gpsimd DMA casts DRAM->DRAM, and the
                # gpsimd queue is already serialized behind the RS, so this
                # touches no other engine (a vector-side cast here stalled
                # the whole pipeline in an earlier revision)
                nc.gpsimd.dma_start(y_d[q * RSH:(q + 1) * RSH, :],
                                    yshards[q][:, :])
    nc.compile()
    return nc


def _get_nc():
    global _CACHED_NC
    if _CACHED_NC is None:
        _CACHED_NC = _build()
    return _CACHED_NC


def _in_maps(x, gate_w, w1, w3, w2):
    import ml_dtypes
    bf = ml_dtypes.bfloat16
    x = np.asarray(x, dtype=np.float32)
    gate_w = np.asarray(gate_w, dtype=np.float32)
    xT = np.ascontiguousarray(x.T)
    xpad = np.zeros((XPAD_ROWS, D), dtype=np.float32)
    xpad[:T] = x

    # host-side capacity check against the actual gate (cheap, exact)
    s = x @ gate_w.T
    thr = np.sort(s, axis=1)[:, -TOPK]          # 2nd-largest score
    routed = s >= thr[:, None]                  # [T, E]
    cnt = routed.reshape(NQ, RT, E).sum(axis=1)  # [NQ, E]
    if cnt.max() > CAP - 8:
        raise RuntimeError(f"routing capacity exceeded: {cnt.max()} > {CAP}-8")

    utri = np.triu(np.ones((P, P), np.float32), k=1)
    ones = np.ones((P, P), np.float32)
    ident = np.eye(P, dtype=np.float32)
    pcol = np.broadcast_to(np.arange(P, dtype=np.float32)[:, None],
                           (P, NF)).astype(bf)
    fcol = np.broadcast_to(np.arange(NF, dtype=np.float32)[None, :],
                           (P, NF)).astype(bf)
    sr = np.broadcast_to(np.arange(CAP, dtype=np.float32)[None, :],
                         (P, CAP)).copy()

    maps = []
    for e in range(NCORES):
        perm = [e] + [j for j in range(E) if j != e]
        # pre-packed [P, DK*E]: gwTr[p, dk*E + e'] = gate_w[perm[e'], dk*128+p]
        gwTr = np.ascontiguousarray(
            gate_w[perm].T.reshape(DK, P, E).transpose(1, 0, 2).reshape(P, DK * E))
        maps.append({
            "xT": xT,
            "x": xpad,
            "gwTr": gwTr,
            "w1T": np.ascontiguousarray(np.asarray(w1[e], np.float32).T).astype(bf),
            "w3T": np.ascontiguousarray(np.asarray(w3[e], np.float32).T).astype(bf),
            "w2T": np.ascontiguousarray(np.asarray(w2[e], np.float32).T).astype(bf),
            "utri": utri,
            "ones": ones,
            "ident": ident,
            "pcol": pcol,
            "fcol": fcol,
            "sr": sr,
        })
    return maps


def run(x, gate_w, w1, w3, w2, trace=False, trace_cores=None):
    nc = _get_nc()
    maps = _in_maps(x, gate_w, w1, w3, w2)
    res = run_bass_kernel_spmd(
        nc, maps, core_ids=list(range(NCORES)), trace=trace,
        trace_cores=trace_cores)
    # core r's output block q (128 rows) holds tokens [1024q + 128r, +128)
    y = np.empty((T, D), dtype=np.float32)
    for r in range(NCORES):
        yr = res.results[r]["y"]
        for q in range(NQ):
            t0 = q * RT + r * RSH
            y[t0:t0 + RSH] = yr[q * RSH:(q + 1) * RSH]
    return y, res


def kernel(x, gate_w, w1, w3, w2):
    y, _ = run(x, gate_w, w1, w3, w2, trace=False)
    return y.astype(np.float32)
